# revision 18
# baseline (speedup 1.0000x reference)
"""MetaPathTransformer Trainium2 kernel (8 NeuronCores, Bass/Tile).

Math: the reference computes heads = inv(D) @ (M0@M1@M2@M3) @ V per
(head, batch), with M_i = sum_a soft[h,a,i] * adjacency[b,a] and D the
(diagonal-by-construction) degree matrix.  The chain is reassociated
right-to-left so every step is [N,N]@[N,256] instead of [N,N]@[N,N]:
per step, T' = sum_a c[s,a,f] * (A_a @ T), evaluated as 9 PE products
mixed on DVE.

PE orientation: T chunk-pairs are the STATIONARY operand and the
SBUF-resident A^T chunks are the MOVING operand, so each stationary
load is reused across all 9 relations (matmuls 2..9 of a PSUM group
carry ldweights=False).  Chain matmuls run in fp8 e4m3 DoubleRow (two
128-row contraction sub-tiles per instruction, 2x the bf16 PE rate);
adjacency is pre-scaled x512 and T re-scaled per step to sit in fp8's
dynamic range, with the scale ratios folded into the mixing
coefficients.  The product (A_a @ T)^T lands with the feature axis on
partitions, so the relation mix is a single fused scalar_tensor_tensor
per product, and the final step's output is exactly attn^T — the W0
matmul of the tail consumes it with no transpose anywhere in the chain.
End-to-end rel err ~2.5e-3 vs the f32 reference (chain quantization is
strongly attenuated by the large residual x in the output).

Sharding (8 cores): core c -> (b = c>>2, q = c&3): batch x n-quarter.
Each core holds A^T[b, :, :, q-slice] fp8 (2.25MB, SBUF-resident) and
computes all 8 heads (full 256-wide f) for its 256-row quarter.  The
per-step exchange is split by feature half: each 128-feature half of
the step's output is transposed back to row-major, quantized to fp8
(32KB) and AllGathered within the 4-core batch group while the other
half computes; the next step's feature-half passes each depend only on
their own half's gather.  A dummy warmup collective runs under the
input DMA to absorb the CC-core ramp.  inv(degree) is folded into the
W0 residual add as a per-partition scale.
"""

import sys

try:
    import concourse.bass as bass  # noqa: F401
except ImportError:  # pragma: no cover
    for _p in ("/opt/trn_rl_repo", "/root/.axon_site/_ro/trn_rl_repo"):
        if _p not in sys.path:
            sys.path.insert(0, _p)
    import concourse.bass as bass  # noqa: F401

import numpy as np
import ml_dtypes

import concourse.mybir as mybir
import concourse.tile as tile
from concourse import bacc
from concourse.bass_utils import run_bass_kernel_spmd

B, A, N, P, D, H = 2, 9, 1024, 4, 256, 8
DH = D // H
EPS = 1e-12
NCORES = 8
NQ = N // 4          # n-quarter per core
MC = N // 128        # n row-chunks

F32 = mybir.dt.float32
F32R = mybir.dt.float32r
BF16 = mybir.dt.bfloat16
F8 = mybir.dt.float8e4
ALU = mybir.AluOpType
ACTF = mybir.ActivationFunctionType
NPBF16 = ml_dtypes.bfloat16
NPF8 = ml_dtypes.float8_e4m3

ASCALE = 512.0                       # adjacency fp8 pre-scale
TSCALE = [1.0, 512.0, 512.0, 512.0]  # T fp8 storage scale per step

_CACHE: dict = {}


def _build_nc(chain_mode: str = "dr8", shared_exout: bool = False, no_reuse: bool = False):
    """chain_mode: 'dr8' fp8+DoubleRow, 'f8' fp8 plain, 'bf16' bf16 chain."""
    fp8 = chain_mode in ("dr8", "f8")
    CDT = F8 if fp8 else BF16
    NPC = NPF8 if fp8 else NPBF16
    nc = bacc.Bacc("TRN2", target_bir_lowering=False, debug=False, num_devices=NCORES)

    dp = nc.declare_dram_parameter
    at_in = dp("at", [A, 128, MC * NQ], CDT, isOutput=False)      # A^T chunk-packed
    xt_in = dp("xt", [2, 128, N], BF16, isOutput=False)           # x[b]^T, d-chunked
    wv_in = dp("wv", [2, 128, D], BF16, isOutput=False)           # Wv_cat (all heads)
    bv_in = dp("bv", [MC, 128, D], BF16, isOutput=False)          # Bv_cat
    cfp_in = dp("cfp", [128, 2, P, A], F32, isOutput=False)       # per-partition coefs
    w0_in = dp("w0", [2, 128, D], BF16, isOutput=False)
    w1_in = dp("w1", [2, 128, 2 * D], BF16, isOutput=False)
    w2_in = dp("w2", [4, 128, D], BF16, isOutput=False)
    xtail_in = dp("xtail", [2, 128, D], F32, isOutput=False)      # x rows of quarter
    invd_in = dp("invd", [128, 2], F32, isOutput=False)           # inv degree cols
    g2_in = dp("g2", [128, D], F32, isOutput=False)
    b2_in = dp("b2", [128, D], F32, isOutput=False)
    gf_in = dp("gf", [128, 2 * D], F32, isOutput=False)
    bf_in = dp("bf", [128, 2 * D], F32, isOutput=False)
    b1_in = dp("b1", [128, 2 * D], F32, isOutput=False)
    b2f_in = dp("b2f", [128, D], F32, isOutput=False)
    id_in = dp("ident", [128, 128], BF16, isOutput=False)
    out_p = dp("out", [2, 128, D], F32, isOutput=True)

    ag4 = [[4 * g + i for i in range(4)] for g in range(NCORES // 4)]
    DR = mybir.MatmulPerfMode.DoubleRow
    WAVES = ((0, 3), (3, 6), (6, A))

    def ex_dram(name, shape):
        if shared_exout:
            return nc.dram_tensor(name, shape, CDT, addr_space="Shared")
        return nc.dram_tensor(name, shape, CDT)

    # The ldweights=False weight-reuse groups require the PE queue to
    # execute in exactly the emitted order (a reordered transpose or
    # matmul would clobber the loaded stationary).  Tile's scheduler
    # can reorder within an engine around stalls, so every PE
    # instruction carries an explicit ordering edge to its predecessor.
    _pe_prev = [None]

    def pe(mm):
        if _pe_prev[0] is not None:
            bass._add_dep_helper(mm.ins, _pe_prev[0].ins, sync=False,
                                 reason="pe-order")
        _pe_prev[0] = mm
        return mm

    with tile.TileContext(nc) as tc:
        with (
            tc.tile_pool(name="atp", bufs=A) as atp,
            tc.tile_pool(name="cst", bufs=1) as cst,
            tc.tile_pool(name="wrk", bufs=1) as wrk,
            tc.tile_pool(name="tt", bufs=2) as tt,
            tc.tile_pool(name="ps", bufs=6, space="PSUM") as ps,
            tc.tile_pool(name="psb", bufs=1, space="PSUM") as psb,
            tc.tile_pool(name="tp", bufs=1, space="PSUM") as tp,
        ):
            # ---- first wave of adjacency, then the V-proj inputs ----
            at8 = [None] * A
            for a in range(3):
                t = atp.tile([128, MC, NQ], CDT, tag="AT", name=f"at8{a}")
                nc.sync.dma_start(t[:].rearrange("p m q -> p (m q)"), at_in[a])
                at8[a] = t
            xt = cst.tile([128, 2, N], BF16)
            nc.sync.dma_start(xt[:], xt_in.rearrange("c p f -> p c f"))
            wv = cst.tile([128, 2, D], BF16)
            nc.sync.dma_start(wv[:], wv_in.rearrange("c p f -> p c f"))
            cfp = cst.tile([128, 2, P, A], F32)
            nc.sync.dma_start(cfp[:], cfp_in[:])
            identb = cst.tile([128, 128], BF16)
            nc.sync.dma_start(identb[:], id_in[:])
            bv = cst.tile([128, MC, D], BF16)
            for m in range(MC):
                nc.sync.dma_start(bv[:, m, :], bv_in[m])

            # warmup collective under the input DMA: absorbs the CC-core
            # ramp AND (via the barrier gate below) the per-core host
            # dispatch skew, which otherwise lands on the first real
            # AllGather of the chain.
            wu = wrk.tile([128, 16], F32, tag="wu")
            nc.vector.memset(wu[:], 0.0)
            wu_in = nc.dram_tensor("wu_in", [128, 16], F32)
            wu_out = nc.dram_tensor("wu_out", [4, 128, 16], F32)
            nc.sync.dma_start(wu_in[:], wu[:])
            nc.gpsimd.collective_compute(
                "AllGather", ALU.bypass, replica_groups=ag4,
                ins=[wu_in[:].opt()], outs=[wu_out[:].opt()])
            wuz = wrk.tile([128, 1], F32, tag="wuz")
            nc.sync.dma_start(wuz[:], wu_out[0, :, 0:1])

            # ---- rest of adjacency (SBUF-resident all 4 steps) ----
            for a in range(3, A):
                t = atp.tile([128, MC, NQ], CDT, tag="AT", name=f"at8{a}")
                nc.sync.dma_start(t[:].rearrange("p m q -> p (m q)"), at_in[a])
                at8[a] = t

            # ---- remaining constants ----
            w0 = cst.tile([128, 2, D], BF16)
            nc.sync.dma_start(w0[:], w0_in.rearrange("c p f -> p c f"))
            w1 = cst.tile([128, 2, 2 * D], BF16)
            nc.sync.dma_start(w1[:], w1_in.rearrange("c p f -> p c f"))
            w2 = cst.tile([128, 4, D], BF16)
            nc.sync.dma_start(w2[:], w2_in.rearrange("c p f -> p c f"))
            xtl = cst.tile([128, 2, D], F32)
            nc.sync.dma_start(xtl[:], xtail_in.rearrange("c p f -> p c f"))
            invd = cst.tile([128, 2], F32)
            nc.sync.dma_start(invd[:], invd_in[:])
            g2b = cst.tile([128, D], F32)
            nc.sync.dma_start(g2b[:], g2_in[:])
            b2b = cst.tile([128, D], F32)
            nc.sync.dma_start(b2b[:], b2_in[:])
            gfb = cst.tile([128, 2 * D], F32)
            nc.sync.dma_start(gfb[:], gf_in[:])
            bfb = cst.tile([128, 2 * D], F32)
            nc.sync.dma_start(bfb[:], bf_in[:])
            b1b = cst.tile([128, 2 * D], F32)
            nc.sync.dma_start(b1b[:], b1_in[:])
            b2fb = cst.tile([128, D], F32)
            nc.sync.dma_start(b2fb[:], b2f_in[:])
            epst = cst.tile([128, 1], F32)
            nc.vector.memset(epst[:], EPS)

            # ---- V = x @ Wv_cat + Bv -> T0 chunks ([n, f], fp8) ----
            # T layout is feature-half-major: [p, fc, chunk, 128f], so the
            # post-AllGather scatter lands as one DMA with 1KB lines.
            tcur = tt.tile([128, 2, MC, 128], CDT, tag="T")
            for m in range(MC):
                pv = ps.tile([128, D], F32, tag="pa")
                for dc in range(2):
                    pe(nc.tensor.matmul(
                        pv[:], xt[:, dc, m * 128:(m + 1) * 128],
                        wv[:, dc, :], start=(dc == 0), stop=(dc == 1)))
                # fp8 quantize fuses the Bv add; chunk 0 also adds the
                # (all-zero) warmup-gather scalar, so the whole PE-ordered
                # chain starts synchronized across the group
                for fc in range(2):
                    fsl = slice(fc * 128, (fc + 1) * 128)
                    if m == 0:
                        nc.vector.scalar_tensor_tensor(
                            tcur[:, fc, m, :], pv[:, fsl], wuz[:, 0:1],
                            bv[:, m, fsl], op0=ALU.add, op1=ALU.add)
                    else:
                        nc.vector.tensor_add(tcur[:, fc, m, :], pv[:, fsl],
                                             bv[:, m, fsl])

            # ---- chain: 4 steps of T <- sum_a cmix[s,a] * (A_a @ T) ----
            def emit_wave(s, fc, a_lo, a_hi, acc_e, acc_o, tcur):
                pas = {}
                if chain_mode == "dr8":
                    for k in range(MC // 2):
                        for a in range(a_lo, a_hi):
                            if k == 0:
                                pas[a] = ps.tile([128, NQ], F32, tag="pa",
                                                 name=f"pa{s}{fc}{a}")
                            mm = pe(nc.tensor.matmul(
                                pas[a][:], tcur[:, fc, 2 * k:2 * k + 2, :],
                                at8[a][:, 2 * k:2 * k + 2, :],
                                start=(k == 0), stop=(k == MC // 2 - 1),
                                perf_mode=DR))
                            if a != a_lo and not no_reuse:
                                mm.ins.ldweights = False
                else:
                    for k in range(MC):
                        for a in range(a_lo, a_hi):
                            if k == 0:
                                pas[a] = ps.tile([128, NQ], F32, tag="pa",
                                                 name=f"pa{s}{fc}{a}")
                            mm = pe(nc.tensor.matmul(
                                pas[a][:], tcur[:, fc, k, :], at8[a][:, k, :],
                                start=(k == 0), stop=(k == MC - 1)))
                            if a != a_lo and not no_reuse:
                                mm.ins.ldweights = False
                # fused mix: acc += pa * c[s,a,f] (two alternating
                # accumulator chains for DVE ILP)
                for a in range(a_lo, a_hi):
                    acc = acc_e if a % 2 == 0 else acc_o
                    sc = cfp[:, fc, s, a:a + 1]
                    if a < 2:
                        nc.vector.tensor_scalar_mul(acc[:], pas[a][:], sc)
                    else:
                        nc.vector.scalar_tensor_tensor(
                            acc[:], pas[a][:], sc, acc[:],
                            op0=ALU.mult, op1=ALU.add)

            def emit_finish(s, fc, acc_e, acc_o, tnext):
                # combine accumulators; transpose own chunks to row-major,
                # quantize fp8, AllGather this feature half in the group
                accb = wrk.tile([128, NQ], BF16, tag=f"accb{fc}")
                nc.vector.tensor_add(accb[:], acc_e[:], acc_o[:])
                exg = wrk.tile([128, 2, 128], CDT, tag=f"exg{fc}")
                for c in range(2):
                    ptr = tp.tile([128, 128], BF16, tag="tp")
                    pe(nc.tensor.transpose(
                        ptr[:], accb[:, c * 128:(c + 1) * 128], identb[:]))
                    nc.vector.tensor_copy(exg[:, c, :], ptr[:])
                exin = nc.dram_tensor(f"exi{s}{fc}", [128, 2, 128], CDT)
                exout = ex_dram(f"exo{s}{fc}", [4, 128, 2, 128])
                nc.sync.dma_start(exin[:], exg[:])
                nc.gpsimd.collective_compute(
                    "AllGather", ALU.bypass, replica_groups=ag4,
                    ins=[exin[:].opt()], outs=[exout[:].opt()])
                # one scatter DMA: dest [p, slot(g,c), f] is contiguous
                # 1KB per partition in the fc plane of tnext
                nc.sync.dma_start(
                    tnext[:, fc].rearrange("p (g c) f -> p g c f", g=4, c=2),
                    exout[:].rearrange("g p c f -> p g c f"))

            att_t = {}
            for s in range(P):
                last = s == P - 1
                tnext = None if last else tt.tile([128, 2, MC, 128], CDT,
                                                  tag="T")
                accs = {}
                for fc in range(2):
                    accs[fc] = (wrk.tile([128, NQ], F32, tag=f"acce{fc}",
                                         name=f"acce{s}{fc}"),
                                wrk.tile([128, NQ], F32, tag=f"acco{fc}",
                                         name=f"acco{s}{fc}"))
                # PE order: fc0 both waves, fc1 wave1, fc0's transposes
                # (its mix completes under fc1-wave1), fc1 wave2, fc1's
                # transposes — so each feature-half's AllGather launches
                # while the other half still computes.
                for w in WAVES:
                    emit_wave(s, 0, *w, *accs[0], tcur)
                emit_wave(s, 1, *WAVES[0], *accs[1], tcur)
                if last:
                    accb0 = wrk.tile([128, NQ], BF16, tag="accb0")
                    nc.vector.tensor_add(accb0[:], accs[0][0][:],
                                         accs[0][1][:])
                    att_t[0] = accb0
                else:
                    emit_finish(s, 0, *accs[0], tnext)
                for w in WAVES[1:]:
                    emit_wave(s, 1, *w, *accs[1], tcur)
                if last:
                    accb1 = wrk.tile([128, NQ], BF16, tag="accb1")
                    nc.vector.tensor_add(accb1[:], accs[1][0][:],
                                         accs[1][1][:])
                    att_t[1] = accb1
                else:
                    emit_finish(s, 1, *accs[1], tnext)
                    tcur = tnext

            # ---- tail for our 256-row n-quarter (2 chunks of 128) ----
            # att_t[fc] holds attn^T directly: [f-half, own 256 rows].
            # Stages are emitted i0/i1-interleaved so the two row-chunks'
            # serial LN chains overlap on the PE/DVE/Scalar queues.
            pr, resid, hb, ht, pf, g1, f2, f2t = {}, {}, {}, {}, {}, {}, {}, {}
            for i in range(2):
                pr[i] = ps.tile([128, D], F32, tag="pa", name=f"pr{i}")
                for fc in range(2):
                    pe(nc.tensor.matmul(
                        pr[i][:], att_t[fc][:, i * 128:(i + 1) * 128],
                        w0[:, fc, :], start=(fc == 0), stop=(fc == 1)))
            for i in range(2):
                # resid = pr * inv_deg + x   (inv(degree) folded in here)
                resid[i] = wrk.tile([128, D], F32, tag=f"resid{i}",
                                    name=f"resid{i}")
                nc.vector.scalar_tensor_tensor(
                    resid[i][:], pr[i][:], invd[:, i:i + 1], xtl[:, i, :],
                    op0=ALU.mult, op1=ALU.add)
            for i in range(2):
                # h = LayerNorm(resid) * gamma2 + beta2
                st = wrk.tile([128, 6], F32, tag=f"st{i}", name=f"st{i}")
                mv = wrk.tile([128, 2], F32, tag=f"mv{i}", name=f"mv{i}")
                nc.vector.bn_stats(st[:], resid[i][:])
                nc.vector.bn_aggr(mv[:], st[:])
                rstd = wrk.tile([128, 1], F32, tag=f"rstd{i}", name=f"rstd{i}")
                nc.scalar.activation(rstd[:], mv[:, 1:2], ACTF.Sqrt,
                                     bias=epst[:], scale=1.0)
                nc.vector.reciprocal(rstd[:], rstd[:])
                hn = wrk.tile([128, D], F32, tag=f"hn{i}", name=f"hn{i}")
                nc.vector.tensor_scalar(hn[:], resid[i][:], mv[:, 0:1],
                                        rstd[:], op0=ALU.subtract,
                                        op1=ALU.mult)
                nc.vector.tensor_mul(hn[:], hn[:], g2b[:])
                hb[i] = wrk.tile([128, D], BF16, tag=f"hb{i}", name=f"hb{i}")
                nc.vector.tensor_add(hb[i][:], hn[:], b2b[:])
            for i in range(2):
                # h^T for the W1 matmul
                ht[i] = wrk.tile([128, 2, 128], BF16, tag=f"ht{i}",
                                 name=f"ht{i}")
                for dc in range(2):
                    ptr = tp.tile([128, 128], BF16, tag="tp", name=f"tph{i}")
                    pe(nc.tensor.transpose(
                        ptr[:], hb[i][:, dc * 128:(dc + 1) * 128], identb[:]))
                    nc.vector.tensor_copy(ht[i][:, dc, :], ptr[:])
            for i in range(2):
                # f = gelu(h @ W1 + b1), then LayerNorm * gf + bf
                pf[i] = psb.tile([128, 2 * D], F32, tag="pf", name=f"pf{i}")
                for dc in range(2):
                    pe(nc.tensor.matmul(pf[i][:], ht[i][:, dc, :],
                                        w1[:, dc, :], start=(dc == 0),
                                        stop=(dc == 1)))
                f1 = wrk.tile([128, 2 * D], F32, tag=f"f1{i}", name=f"f1{i}")
                nc.vector.tensor_add(f1[:], pf[i][:], b1b[:])
                g1[i] = wrk.tile([128, 2 * D], F32, tag=f"g1{i}",
                                 name=f"g1{i}")
                nc.scalar.activation(g1[i][:], f1[:], ACTF.Gelu)
            for i in range(2):
                st2 = wrk.tile([128, 6], F32, tag=f"st2{i}", name=f"st2{i}")
                mv2 = wrk.tile([128, 2], F32, tag=f"mv2{i}", name=f"mv2{i}")
                nc.vector.bn_stats(st2[:], g1[i][:])
                nc.vector.bn_aggr(mv2[:], st2[:])
                rstd2 = wrk.tile([128, 1], F32, tag=f"rstd2{i}",
                                 name=f"rstd2{i}")
                nc.scalar.activation(rstd2[:], mv2[:, 1:2], ACTF.Sqrt,
                                     bias=epst[:], scale=1.0)
                nc.vector.reciprocal(rstd2[:], rstd2[:])
                fn = wrk.tile([128, 2 * D], F32, tag=f"fn{i}", name=f"fn{i}")
                nc.vector.tensor_scalar(fn[:], g1[i][:], mv2[:, 0:1],
                                        rstd2[:], op0=ALU.subtract,
                                        op1=ALU.mult)
                nc.vector.tensor_mul(fn[:], fn[:], gfb[:])
                f2[i] = wrk.tile([128, 2 * D], BF16, tag=f"f2{i}",
                                 name=f"f2{i}")
                nc.vector.tensor_add(f2[i][:], fn[:], bfb[:])
            for i in range(2):
                # f2^T, then out = f2 @ W2 + b2f + resid
                f2t[i] = wrk.tile([128, 4, 128], BF16, tag=f"f2t{i}",
                                  name=f"f2t{i}")
                for k in range(4):
                    ptr = tp.tile([128, 128], BF16, tag="tp", name=f"tpf{i}")
                    pe(nc.tensor.transpose(
                        ptr[:], f2[i][:, k * 128:(k + 1) * 128], identb[:]))
                    nc.vector.tensor_copy(f2t[i][:, k, :], ptr[:])
            for i in range(2):
                po = ps.tile([128, D], F32, tag="pa", name=f"po{i}")
                for k in range(4):
                    pe(nc.tensor.matmul(po[:], f2t[i][:, k, :], w2[:, k, :],
                                        start=(k == 0), stop=(k == 3)))
                ot = wrk.tile([128, D], F32, tag=f"ot{i}", name=f"ot{i}")
                nc.vector.tensor_add(ot[:], po[:], b2fb[:])
                nc.vector.tensor_add(ot[:], ot[:], resid[i][:])
                nc.sync.dma_start(out_p[i], ot[:])

    nc.finalize()
    return nc


def _softmax_relu(kernels):
    r = np.maximum(kernels, 0.0)
    e = np.exp(r - r.max(axis=1, keepdims=True))
    return (e / e.sum(axis=1, keepdims=True)).astype(np.float32)  # [H, A, P]


def _prep_in_maps(adjacency, degree, x, kernels, Wv, Bv, W0, gamma2, beta2,
                  W1, b1, gf, bf, W2, b2f, chain_mode: str = "dr8"):
    fp8 = chain_mode in ("dr8", "f8")
    cdt = NPF8 if fp8 else NPBF16
    ascale = ASCALE if fp8 else 1.0
    tsc = TSCALE if fp8 else [1.0] * P

    soft = _softmax_relu(np.asarray(kernels, np.float32))
    wv_cat = np.ascontiguousarray(
        np.transpose(np.asarray(Wv, np.float32), (1, 0, 2)).reshape(D, D))
    bv_cat = np.transpose(np.asarray(Bv, np.float32), (1, 0, 2)).reshape(N, D)
    bv_r = (bv_cat.reshape(MC, 128, D) * tsc[0]).astype(NPBF16)
    invd_full = 1.0 / np.diagonal(np.asarray(degree, np.float32),
                                  axis1=1, axis2=2)  # [B, N]
    eye = np.eye(128, dtype=NPBF16)
    ones128 = np.ones((128, 1), np.float32)

    g2 = ones128 * np.asarray(gamma2, np.float32)[None, :]
    b2 = ones128 * np.asarray(beta2, np.float32)[None, :]
    gfB = ones128 * np.asarray(gf, np.float32)[None, :]
    bfB = ones128 * np.asarray(bf, np.float32)[None, :]
    b1B = ones128 * np.asarray(b1, np.float32)[None, :]
    b2fB = ones128 * np.asarray(b2f, np.float32)[None, :]
    w0r = np.asarray(W0, np.float32).reshape(2, 128, D).astype(NPBF16)
    w1r = np.asarray(W1, np.float32).reshape(2, 128, 2 * D).astype(NPBF16)
    w2r = np.asarray(W2, np.float32).reshape(4, 128, D).astype(NPBF16)

    # mix coefficients: chain step s applies soft[:, :, P-1-s]; fold the
    # adjacency fp8 pre-scale and the per-step T storage scales in.  The
    # final step folds T's storage scale out (output at true scale).
    hidx = np.arange(D) // DH
    tsc_out = list(tsc[1:]) + [1.0]
    cmix = np.empty((P, A, D), np.float32)
    for s in range(P):
        cmix[s] = (soft[hidx, :, P - 1 - s].T
                   * (tsc_out[s] / (tsc[s] * ascale)))
    # per-partition layout: cfp[p, fc, s, a] = cmix[s, a, fc*128 + p]
    cfp = np.ascontiguousarray(
        cmix.transpose(2, 0, 1).reshape(2, 128, P, A).transpose(1, 0, 2, 3))

    adjacency = np.asarray(adjacency, np.float32)
    x = np.asarray(x, np.float32)

    in_maps = []
    for c in range(NCORES):
        b = c >> 2
        q = c & 3
        nsl = slice(q * NQ, (q + 1) * NQ)

        # at[a, p, m*NQ+j] = adjacency[b, a, q*NQ+j, m*128+p] * ascale
        at_t = adjacency[b].transpose(0, 2, 1)[:, :, nsl]       # [A, n, NQ]
        at_c = np.ascontiguousarray(
            at_t.reshape(A, MC, 128, NQ).transpose(0, 2, 1, 3)
            .reshape(A, 128, MC * NQ)) * ascale
        xt_c = np.ascontiguousarray(x[b].T).reshape(2, 128, N).astype(NPBF16)
        xtail_c = np.ascontiguousarray(x[b, nsl]).reshape(2, 128, D)
        invd_c = np.ascontiguousarray(
            invd_full[b, nsl].reshape(2, 128).T)                 # [128, 2]

        in_maps.append({
            "at": at_c.astype(cdt),
            "xt": xt_c,
            "wv": wv_cat.reshape(2, 128, D).astype(NPBF16),
            "bv": bv_r,
            "cfp": cfp,
            "w0": w0r, "w1": w1r, "w2": w2r,
            "xtail": xtail_c,
            "invd": invd_c.astype(np.float32),
            "g2": g2, "b2": b2, "gf": gfB, "bf": bfB, "b1": b1B, "b2f": b2fB,
            "ident": eye,
        })
    return in_maps


def kernel(**inputs) -> np.ndarray:
    if "nc" not in _CACHE:
        _CACHE["nc"] = _build_nc()
    nc = _CACHE["nc"]
    in_maps = _prep_in_maps(**inputs)
    res = run_bass_kernel_spmd(nc, in_maps, core_ids=list(range(NCORES)))
    out = np.empty((B, N, D), np.float32)
    for c in range(NCORES):
        b, q = c >> 2, c & 3
        out[b, q * NQ:(q + 1) * NQ] = res.results[c]["out"].reshape(NQ, D)
    return out


# revision 21
# speedup vs baseline: 1.1459x; 1.1459x over previous
"""MetaPathTransformer Trainium2 kernel (8 NeuronCores, Bass/Tile).

Math: the reference computes heads = inv(D) @ (M0@M1@M2@M3) @ V per
(head, batch), with M_i = sum_a soft[h,a,i] * adjacency[b,a] and D the
(diagonal-by-construction) degree matrix.  The chain is reassociated
right-to-left so every step is [N,N]@[N,256] instead of [N,N]@[N,N]:
per step, T' = sum_a c[s,a,f] * (A_a @ T), evaluated as 9 PE products
mixed on DVE.

PE orientation: T chunk-pairs are the STATIONARY operand and the
SBUF-resident A^T chunks are the MOVING operand, so each stationary
load is reused across all 9 relations (matmuls 2..9 of a PSUM group
carry ldweights=False).  Chain matmuls run in fp8 e4m3 DoubleRow (two
128-row contraction sub-tiles per instruction, 2x the bf16 PE rate);
adjacency is pre-scaled x512 and T re-scaled per step to sit in fp8's
dynamic range, with the scale ratios folded into the mixing
coefficients.  The product (A_a @ T)^T lands with the feature axis on
partitions, so the relation mix is a single fused scalar_tensor_tensor
per product, and the final step's output is exactly attn^T — the W0
matmul of the tail consumes it with no transpose anywhere in the chain.
End-to-end rel err ~2.5e-3 vs the f32 reference (chain quantization is
strongly attenuated by the large residual x in the output).

Sharding (8 cores): core c -> (b = c>>2, q = c&3): batch x n-quarter.
Each core holds A^T[b, :, :, q-slice] fp8 (2.25MB, SBUF-resident) and
computes all 8 heads (full 256-wide f) for its 256-row quarter.  The
per-step exchange is split by feature half: each 128-feature half of
the step's output is transposed back to row-major, quantized to fp8
(32KB) and AllGathered within the 4-core batch group while the other
half computes; the next step's feature-half passes each depend only on
their own half's gather.  A dummy warmup collective runs under the
input DMA to absorb the CC-core ramp.  inv(degree) is folded into the
W0 residual add as a per-partition scale.
"""

import sys

try:
    import concourse.bass as bass  # noqa: F401
except ImportError:  # pragma: no cover
    for _p in ("/opt/trn_rl_repo", "/root/.axon_site/_ro/trn_rl_repo"):
        if _p not in sys.path:
            sys.path.insert(0, _p)
    import concourse.bass as bass  # noqa: F401

import numpy as np
import ml_dtypes

import concourse.mybir as mybir
import concourse.tile as tile
from concourse import bacc
from concourse.bass_utils import run_bass_kernel_spmd

B, A, N, P, D, H = 2, 9, 1024, 4, 256, 8
DH = D // H
EPS = 1e-12
NCORES = 8
NQ = N // 4          # n-quarter per core
MC = N // 128        # n row-chunks

F32 = mybir.dt.float32
F32R = mybir.dt.float32r
BF16 = mybir.dt.bfloat16
F8 = mybir.dt.float8e4
ALU = mybir.AluOpType
ACTF = mybir.ActivationFunctionType
NPBF16 = ml_dtypes.bfloat16
NPF8 = ml_dtypes.float8_e4m3

ASCALE = 512.0                       # adjacency fp8 pre-scale
TSCALE = [1.0, 512.0, 512.0, 512.0]  # T fp8 storage scale per step

_CACHE: dict = {}


def _build_nc(chain_mode: str = "dr8", shared_exout: bool = False, no_reuse: bool = False):
    """chain_mode: 'dr8' fp8+DoubleRow, 'f8' fp8 plain, 'bf16' bf16 chain."""
    fp8 = chain_mode in ("dr8", "f8")
    CDT = F8 if fp8 else BF16
    NPC = NPF8 if fp8 else NPBF16
    nc = bacc.Bacc("TRN2", target_bir_lowering=False, debug=False, num_devices=NCORES)

    dp = nc.declare_dram_parameter
    at_in = dp("at", [A, 128, MC * NQ], CDT, isOutput=False)      # A^T chunk-packed
    xt_in = dp("xt", [2, 128, N], BF16, isOutput=False)           # x[b]^T, d-chunked
    wv_in = dp("wv", [2, 128, D], BF16, isOutput=False)           # Wv_cat (all heads)
    bv_in = dp("bv", [MC, 128, D], BF16, isOutput=False)          # Bv_cat
    cfp_in = dp("cfp", [128, 2, P, A], F32, isOutput=False)       # per-partition coefs
    w0_in = dp("w0", [2, 128, D], BF16, isOutput=False)
    w1_in = dp("w1", [2, 128, 2 * D], BF16, isOutput=False)
    w2_in = dp("w2", [4, 128, D], BF16, isOutput=False)
    xtail_in = dp("xtail", [2, 128, D], F32, isOutput=False)      # x rows of quarter
    invd_in = dp("invd", [128, 2], F32, isOutput=False)           # inv degree cols
    g2_in = dp("g2", [128, D], F32, isOutput=False)
    b2_in = dp("b2", [128, D], F32, isOutput=False)
    gf_in = dp("gf", [128, 2 * D], F32, isOutput=False)
    bf_in = dp("bf", [128, 2 * D], F32, isOutput=False)
    b1_in = dp("b1", [128, 2 * D], F32, isOutput=False)
    b2f_in = dp("b2f", [128, D], F32, isOutput=False)
    id_in = dp("ident", [128, 128], BF16, isOutput=False)
    out_p = dp("out", [2, 128, D], F32, isOutput=True)

    ag4 = [[4 * g + i for i in range(4)] for g in range(NCORES // 4)]
    DR = mybir.MatmulPerfMode.DoubleRow
    WAVES = ((0, 3), (3, 6), (6, A))

    def ex_dram(name, shape):
        if shared_exout:
            return nc.dram_tensor(name, shape, CDT, addr_space="Shared")
        return nc.dram_tensor(name, shape, CDT)

    # The ldweights=False weight-reuse groups require the PE queue to
    # execute in exactly the emitted order (a reordered transpose or
    # matmul would clobber the loaded stationary).  Tile's scheduler
    # can reorder within an engine around stalls, so every PE
    # instruction carries an explicit ordering edge to its predecessor.
    _pe_prev = [None]

    def pe(mm):
        if _pe_prev[0] is not None:
            bass._add_dep_helper(mm.ins, _pe_prev[0].ins, sync=False,
                                 reason="pe-order")
        _pe_prev[0] = mm
        return mm

    with tile.TileContext(nc) as tc:
        with (
            tc.tile_pool(name="atp", bufs=A) as atp,
            tc.tile_pool(name="cst", bufs=1) as cst,
            tc.tile_pool(name="wrk", bufs=1) as wrk,
            tc.tile_pool(name="tt", bufs=2) as tt,
            tc.tile_pool(name="ps", bufs=5, space="PSUM") as ps,
            tc.tile_pool(name="psb", bufs=1, space="PSUM") as psb,
            tc.tile_pool(name="tp", bufs=2, space="PSUM") as tp,
        ):
            # warmup collective, issued before everything else: pays the
            # first-collective setup/ramp cost under the input DMA phase
            # instead of on the chain's first real AllGather.
            wu = wrk.tile([128, 16], F32, tag="wu")
            nc.vector.memset(wu[:], 0.0)
            wu_in = nc.dram_tensor("wu_in", [128, 16], F32)
            wu_out = nc.dram_tensor("wu_out", [4, 128, 16], F32)
            nc.sync.dma_start(wu_in[:], wu[:])
            nc.gpsimd.collective_compute(
                "AllGather", ALU.bypass, replica_groups=ag4,
                ins=[wu_in[:].opt()], outs=[wu_out[:].opt()])

            # ---- first wave of adjacency, then the V-proj inputs ----
            at8 = [None] * A
            for a in range(3):
                t = atp.tile([128, MC, NQ], CDT, tag="AT", name=f"at8{a}")
                nc.sync.dma_start(t[:].rearrange("p m q -> p (m q)"), at_in[a])
                at8[a] = t
            xt = cst.tile([128, 2, N], BF16)
            nc.sync.dma_start(xt[:], xt_in.rearrange("c p f -> p c f"))
            wv = cst.tile([128, 2, D], BF16)
            nc.sync.dma_start(wv[:], wv_in.rearrange("c p f -> p c f"))
            cfp = cst.tile([128, 2, P, A], F32)
            nc.sync.dma_start(cfp[:], cfp_in[:])
            identb = cst.tile([128, 128], BF16)
            nc.sync.dma_start(identb[:], id_in[:])
            bv = cst.tile([128, MC, D], BF16)
            for m in range(MC):
                nc.sync.dma_start(bv[:, m, :], bv_in[m])


            # ---- rest of adjacency (SBUF-resident all 4 steps) ----
            for a in range(3, A):
                t = atp.tile([128, MC, NQ], CDT, tag="AT", name=f"at8{a}")
                nc.sync.dma_start(t[:].rearrange("p m q -> p (m q)"), at_in[a])
                at8[a] = t

            # ---- remaining constants ----
            w0 = cst.tile([128, 2, D], BF16)
            nc.sync.dma_start(w0[:], w0_in.rearrange("c p f -> p c f"))
            w1 = cst.tile([128, 2, 2 * D], BF16)
            nc.sync.dma_start(w1[:], w1_in.rearrange("c p f -> p c f"))
            w2 = cst.tile([128, 4, D], BF16)
            nc.sync.dma_start(w2[:], w2_in.rearrange("c p f -> p c f"))
            xtl = cst.tile([128, 2, D], F32)
            nc.sync.dma_start(xtl[:], xtail_in.rearrange("c p f -> p c f"))
            invd = cst.tile([128, 2], F32)
            nc.sync.dma_start(invd[:], invd_in[:])
            g2b = cst.tile([128, D], F32)
            nc.sync.dma_start(g2b[:], g2_in[:])
            b2b = cst.tile([128, D], F32)
            nc.sync.dma_start(b2b[:], b2_in[:])
            gfb = cst.tile([128, 2 * D], F32)
            nc.sync.dma_start(gfb[:], gf_in[:])
            bfb = cst.tile([128, 2 * D], F32)
            nc.sync.dma_start(bfb[:], bf_in[:])
            b1b = cst.tile([128, 2 * D], F32)
            nc.sync.dma_start(b1b[:], b1_in[:])
            b2fb = cst.tile([128, D], F32)
            nc.sync.dma_start(b2fb[:], b2f_in[:])
            epst = cst.tile([128, 1], F32)
            nc.vector.memset(epst[:], EPS)

            # ---- V = x @ Wv_cat + Bv -> T0 chunks ([n, f], fp8) ----
            # T layout is feature-half-major: [p, fc, chunk, 128f], so the
            # post-AllGather scatter lands as one DMA with 1KB lines.
            tcur = tt.tile([128, 2, MC, 128], CDT, tag="T")
            for m in range(MC):
                pv = ps.tile([128, D], F32, tag="pa")
                for dc in range(2):
                    pe(nc.tensor.matmul(
                        pv[:], xt[:, dc, m * 128:(m + 1) * 128],
                        wv[:, dc, :], start=(dc == 0), stop=(dc == 1)))
                # fp8 quantize fuses the Bv add
                for fc in range(2):
                    fsl = slice(fc * 128, (fc + 1) * 128)
                    nc.vector.tensor_add(tcur[:, fc, m, :], pv[:, fsl],
                                         bv[:, m, fsl])

            # ---- chain: 4 steps of T <- sum_a cmix[s,a] * (A_a @ T) ----
            def emit_wave(s, fc, a_lo, a_hi, acc_e, acc_o, tcur):
                pas = {}
                if chain_mode == "dr8":
                    for k in range(MC // 2):
                        for a in range(a_lo, a_hi):
                            if k == 0:
                                pas[a] = ps.tile([128, NQ], F32, tag="pa",
                                                 name=f"pa{s}{fc}{a}")
                            mm = pe(nc.tensor.matmul(
                                pas[a][:], tcur[:, fc, 2 * k:2 * k + 2, :],
                                at8[a][:, 2 * k:2 * k + 2, :],
                                start=(k == 0), stop=(k == MC // 2 - 1),
                                perf_mode=DR))
                            if a != a_lo and not no_reuse:
                                mm.ins.ldweights = False
                else:
                    for k in range(MC):
                        for a in range(a_lo, a_hi):
                            if k == 0:
                                pas[a] = ps.tile([128, NQ], F32, tag="pa",
                                                 name=f"pa{s}{fc}{a}")
                            mm = pe(nc.tensor.matmul(
                                pas[a][:], tcur[:, fc, k, :], at8[a][:, k, :],
                                start=(k == 0), stop=(k == MC - 1)))
                            if a != a_lo and not no_reuse:
                                mm.ins.ldweights = False
                # fused mix: acc += pa * c[s,a,f] (two alternating
                # accumulator chains for DVE ILP)
                for a in range(a_lo, a_hi):
                    acc = acc_e if a % 2 == 0 else acc_o
                    sc = cfp[:, fc, s, a:a + 1]
                    if a < 2:
                        nc.vector.tensor_scalar_mul(acc[:], pas[a][:], sc)
                    else:
                        nc.vector.scalar_tensor_tensor(
                            acc[:], pas[a][:], sc, acc[:],
                            op0=ALU.mult, op1=ALU.add)

            def emit_finish(s, fc, acc_e, acc_o, tnext):
                # combine accumulators (bf16), transpose own chunks back to
                # row-major, quantize to fp8 on the Scalar engine
                accb = wrk.tile([128, NQ], BF16, tag=f"accb{fc}")
                nc.vector.tensor_add(accb[:], acc_e[:], acc_o[:])
                exg = wrk.tile([128, 2, 128], CDT, tag=f"exg{fc}")
                for c in range(2):
                    ptr = tp.tile([128, 128], BF16, tag="tp", name=f"ptr{fc}")
                    pe(nc.tensor.transpose(
                        ptr[:], accb[:, c * 128:(c + 1) * 128], identb[:]))
                    nc.scalar.activation(exg[:, c, :], ptr[:], ACTF.Copy)
                exin = nc.dram_tensor(f"exi{s}{fc}", [128, 2, 128], CDT)
                exout = ex_dram(f"exo{s}{fc}", [4, 128, 2, 128])
                nc.sync.dma_start(exin[:], exg[:])
                nc.gpsimd.collective_compute(
                    "AllGather", ALU.bypass, replica_groups=ag4,
                    ins=[exin[:].opt()], outs=[exout[:].opt()])
                # one scatter DMA: dest [p, slot(g,c), f] is contiguous
                # 1KB per partition in the fc plane of tnext
                nc.sync.dma_start(
                    tnext[:, fc].rearrange("p (g c) f -> p g c f", g=4, c=2),
                    exout[:].rearrange("g p c f -> p g c f"))

            att_t = {}
            for s in range(P):
                last = s == P - 1
                tnext = None if last else tt.tile([128, 2, MC, 128], CDT,
                                                  tag="T")
                accs = {}
                for fc in range(2):
                    accs[fc] = (wrk.tile([128, NQ], F32, tag=f"acce{fc}",
                                         name=f"acce{s}{fc}"),
                                wrk.tile([128, NQ], F32, tag=f"acco{fc}",
                                         name=f"acco{s}{fc}"))
                # PE order: fc0 both waves, fc1 wave1, fc0's transposes
                # (its mix completes under fc1-wave1), fc1 wave2, fc1's
                # transposes — so each feature-half's AllGather launches
                # while the other half still computes.
                for w in WAVES:
                    emit_wave(s, 0, *w, *accs[0], tcur)
                emit_wave(s, 1, *WAVES[0], *accs[1], tcur)
                if last:
                    accb0 = wrk.tile([128, NQ], BF16, tag="accb0")
                    nc.vector.tensor_add(accb0[:], accs[0][0][:],
                                         accs[0][1][:])
                    att_t[0] = accb0
                else:
                    emit_finish(s, 0, *accs[0], tnext)
                for w in WAVES[1:]:
                    emit_wave(s, 1, *w, *accs[1], tcur)
                if last:
                    accb1 = wrk.tile([128, NQ], BF16, tag="accb1")
                    nc.vector.tensor_add(accb1[:], accs[1][0][:],
                                         accs[1][1][:])
                    att_t[1] = accb1
                else:
                    emit_finish(s, 1, *accs[1], tnext)
                    tcur = tnext

            # ---- tail for our 256-row n-quarter (2 chunks of 128) ----
            # att_t[fc] holds attn^T directly: [f-half, own 256 rows].
            # Stages are emitted i0/i1-interleaved so the two row-chunks'
            # serial LN chains overlap on the PE/DVE/Scalar queues.
            pr, resid, hb, ht, pf, g1, f2, f2t = {}, {}, {}, {}, {}, {}, {}, {}
            for i in range(2):
                pr[i] = ps.tile([128, D], F32, tag="pa", name=f"pr{i}")
                for fc in range(2):
                    pe(nc.tensor.matmul(
                        pr[i][:], att_t[fc][:, i * 128:(i + 1) * 128],
                        w0[:, fc, :], start=(fc == 0), stop=(fc == 1)))
            for i in range(2):
                # resid = pr * inv_deg + x   (inv(degree) folded in here)
                resid[i] = wrk.tile([128, D], F32, tag=f"resid{i}",
                                    name=f"resid{i}")
                nc.vector.scalar_tensor_tensor(
                    resid[i][:], pr[i][:], invd[:, i:i + 1], xtl[:, i, :],
                    op0=ALU.mult, op1=ALU.add)
            for i in range(2):
                # h = LayerNorm(resid) * gamma2 + beta2
                st = wrk.tile([128, 6], F32, tag=f"st{i}", name=f"st{i}")
                mv = wrk.tile([128, 2], F32, tag=f"mv{i}", name=f"mv{i}")
                nc.vector.bn_stats(st[:], resid[i][:])
                nc.vector.bn_aggr(mv[:], st[:])
                rstd = wrk.tile([128, 1], F32, tag=f"rstd{i}", name=f"rstd{i}")
                nc.scalar.activation(rstd[:], mv[:, 1:2], ACTF.Sqrt,
                                     bias=epst[:], scale=1.0)
                nc.vector.reciprocal(rstd[:], rstd[:])
                hn = wrk.tile([128, D], F32, tag=f"hn{i}", name=f"hn{i}")
                nc.vector.tensor_scalar(hn[:], resid[i][:], mv[:, 0:1],
                                        rstd[:], op0=ALU.subtract,
                                        op1=ALU.mult)
                nc.gpsimd.tensor_mul(hn[:], hn[:], g2b[:])
                hb[i] = wrk.tile([128, D], BF16, tag=f"hb{i}", name=f"hb{i}")
                nc.gpsimd.tensor_add(hb[i][:], hn[:], b2b[:])
            for i in range(2):
                # h^T for the W1 matmul
                ht[i] = wrk.tile([128, 2, 128], BF16, tag=f"ht{i}",
                                 name=f"ht{i}")
                for dc in range(2):
                    ptr = tp.tile([128, 128], BF16, tag="tp", name=f"tph{i}")
                    pe(nc.tensor.transpose(
                        ptr[:], hb[i][:, dc * 128:(dc + 1) * 128], identb[:]))
                    nc.scalar.activation(ht[i][:, dc, :], ptr[:], ACTF.Copy)
            for i in range(2):
                # f = gelu(h @ W1 + b1), then LayerNorm * gf + bf
                pf[i] = psb.tile([128, 2 * D], F32, tag="pf", name=f"pf{i}")
                for dc in range(2):
                    pe(nc.tensor.matmul(pf[i][:], ht[i][:, dc, :],
                                        w1[:, dc, :], start=(dc == 0),
                                        stop=(dc == 1)))
                f1 = wrk.tile([128, 2 * D], F32, tag=f"f1{i}", name=f"f1{i}")
                nc.vector.tensor_add(f1[:], pf[i][:], b1b[:])
                g1[i] = wrk.tile([128, 2 * D], F32, tag=f"g1{i}",
                                 name=f"g1{i}")
                nc.scalar.activation(g1[i][:], f1[:], ACTF.Gelu)
            for i in range(2):
                st2 = wrk.tile([128, 6], F32, tag=f"st2{i}", name=f"st2{i}")
                mv2 = wrk.tile([128, 2], F32, tag=f"mv2{i}", name=f"mv2{i}")
                nc.vector.bn_stats(st2[:], g1[i][:])
                nc.vector.bn_aggr(mv2[:], st2[:])
                rstd2 = wrk.tile([128, 1], F32, tag=f"rstd2{i}",
                                 name=f"rstd2{i}")
                nc.scalar.activation(rstd2[:], mv2[:, 1:2], ACTF.Sqrt,
                                     bias=epst[:], scale=1.0)
                nc.vector.reciprocal(rstd2[:], rstd2[:])
                fn = wrk.tile([128, 2 * D], F32, tag=f"fn{i}", name=f"fn{i}")
                nc.vector.tensor_scalar(fn[:], g1[i][:], mv2[:, 0:1],
                                        rstd2[:], op0=ALU.subtract,
                                        op1=ALU.mult)
                nc.gpsimd.tensor_mul(fn[:], fn[:], gfb[:])
                f2[i] = wrk.tile([128, 2 * D], BF16, tag=f"f2{i}",
                                 name=f"f2{i}")
                nc.gpsimd.tensor_add(f2[i][:], fn[:], bfb[:])
            for i in range(2):
                # f2^T, then out = f2 @ W2 + b2f + resid
                f2t[i] = wrk.tile([128, 4, 128], BF16, tag=f"f2t{i}",
                                  name=f"f2t{i}")
                for k in range(4):
                    ptr = tp.tile([128, 128], BF16, tag="tp", name=f"tpf{i}")
                    pe(nc.tensor.transpose(
                        ptr[:], f2[i][:, k * 128:(k + 1) * 128], identb[:]))
                    nc.scalar.activation(f2t[i][:, k, :], ptr[:], ACTF.Copy)
            for i in range(2):
                po = ps.tile([128, D], F32, tag="pa", name=f"po{i}")
                for k in range(4):
                    pe(nc.tensor.matmul(po[:], f2t[i][:, k, :], w2[:, k, :],
                                        start=(k == 0), stop=(k == 3)))
                ot = wrk.tile([128, D], F32, tag=f"ot{i}", name=f"ot{i}")
                nc.vector.tensor_add(ot[:], po[:], b2fb[:])
                nc.vector.tensor_add(ot[:], ot[:], resid[i][:])
                nc.sync.dma_start(out_p[i], ot[:])

    nc.finalize()
    return nc


def _softmax_relu(kernels):
    r = np.maximum(kernels, 0.0)
    e = np.exp(r - r.max(axis=1, keepdims=True))
    return (e / e.sum(axis=1, keepdims=True)).astype(np.float32)  # [H, A, P]


def _prep_in_maps(adjacency, degree, x, kernels, Wv, Bv, W0, gamma2, beta2,
                  W1, b1, gf, bf, W2, b2f, chain_mode: str = "dr8"):
    fp8 = chain_mode in ("dr8", "f8")
    cdt = NPF8 if fp8 else NPBF16
    ascale = ASCALE if fp8 else 1.0
    tsc = TSCALE if fp8 else [1.0] * P

    soft = _softmax_relu(np.asarray(kernels, np.float32))
    wv_cat = np.ascontiguousarray(
        np.transpose(np.asarray(Wv, np.float32), (1, 0, 2)).reshape(D, D))
    bv_cat = np.transpose(np.asarray(Bv, np.float32), (1, 0, 2)).reshape(N, D)
    bv_r = (bv_cat.reshape(MC, 128, D) * tsc[0]).astype(NPBF16)
    invd_full = 1.0 / np.diagonal(np.asarray(degree, np.float32),
                                  axis1=1, axis2=2)  # [B, N]
    eye = np.eye(128, dtype=NPBF16)
    ones128 = np.ones((128, 1), np.float32)

    g2 = ones128 * np.asarray(gamma2, np.float32)[None, :]
    b2 = ones128 * np.asarray(beta2, np.float32)[None, :]
    gfB = ones128 * np.asarray(gf, np.float32)[None, :]
    bfB = ones128 * np.asarray(bf, np.float32)[None, :]
    b1B = ones128 * np.asarray(b1, np.float32)[None, :]
    b2fB = ones128 * np.asarray(b2f, np.float32)[None, :]
    w0r = np.asarray(W0, np.float32).reshape(2, 128, D).astype(NPBF16)
    w1r = np.asarray(W1, np.float32).reshape(2, 128, 2 * D).astype(NPBF16)
    w2r = np.asarray(W2, np.float32).reshape(4, 128, D).astype(NPBF16)

    # mix coefficients: chain step s applies soft[:, :, P-1-s]; fold the
    # adjacency fp8 pre-scale and the per-step T storage scales in.  The
    # final step folds T's storage scale out (output at true scale).
    hidx = np.arange(D) // DH
    tsc_out = list(tsc[1:]) + [1.0]
    cmix = np.empty((P, A, D), np.float32)
    for s in range(P):
        cmix[s] = (soft[hidx, :, P - 1 - s].T
                   * (tsc_out[s] / (tsc[s] * ascale)))
    # per-partition layout: cfp[p, fc, s, a] = cmix[s, a, fc*128 + p]
    cfp = np.ascontiguousarray(
        cmix.transpose(2, 0, 1).reshape(2, 128, P, A).transpose(1, 0, 2, 3))

    adjacency = np.asarray(adjacency, np.float32)
    x = np.asarray(x, np.float32)

    in_maps = []
    for c in range(NCORES):
        b = c >> 2
        q = c & 3
        nsl = slice(q * NQ, (q + 1) * NQ)

        # at[a, p, m*NQ+j] = adjacency[b, a, q*NQ+j, m*128+p] * ascale
        at_t = adjacency[b].transpose(0, 2, 1)[:, :, nsl]       # [A, n, NQ]
        at_c = np.ascontiguousarray(
            at_t.reshape(A, MC, 128, NQ).transpose(0, 2, 1, 3)
            .reshape(A, 128, MC * NQ)) * ascale
        xt_c = np.ascontiguousarray(x[b].T).reshape(2, 128, N).astype(NPBF16)
        xtail_c = np.ascontiguousarray(x[b, nsl]).reshape(2, 128, D)
        invd_c = np.ascontiguousarray(
            invd_full[b, nsl].reshape(2, 128).T)                 # [128, 2]

        in_maps.append({
            "at": at_c.astype(cdt),
            "xt": xt_c,
            "wv": wv_cat.reshape(2, 128, D).astype(NPBF16),
            "bv": bv_r,
            "cfp": cfp,
            "w0": w0r, "w1": w1r, "w2": w2r,
            "xtail": xtail_c,
            "invd": invd_c.astype(np.float32),
            "g2": g2, "b2": b2, "gf": gfB, "bf": bfB, "b1": b1B, "b2f": b2fB,
            "ident": eye,
        })
    return in_maps


def kernel(**inputs) -> np.ndarray:
    if "nc" not in _CACHE:
        _CACHE["nc"] = _build_nc()
    nc = _CACHE["nc"]
    in_maps = _prep_in_maps(**inputs)
    res = run_bass_kernel_spmd(nc, in_maps, core_ids=list(range(NCORES)))
    out = np.empty((B, N, D), np.float32)
    for c in range(NCORES):
        b, q = c >> 2, c & 3
        out[b, q * NQ:(q + 1) * NQ] = res.results[c]["out"].reshape(NQ, D)
    return out


# revision 23
# speedup vs baseline: 1.1637x; 1.0156x over previous
"""MetaPathTransformer Trainium2 kernel (8 NeuronCores, Bass/Tile).

Math: the reference computes heads = inv(D) @ (M0@M1@M2@M3) @ V per
(head, batch), with M_i = sum_a soft[h,a,i] * adjacency[b,a] and D the
(diagonal-by-construction) degree matrix.  The chain is reassociated
right-to-left so every step is [N,N]@[N,256] instead of [N,N]@[N,N]:
per step, T' = sum_a c[s,a,f] * (A_a @ T), evaluated as 9 PE products
mixed on DVE.

PE orientation: T chunk-pairs are the STATIONARY operand and the
SBUF-resident A^T chunks are the MOVING operand, so each stationary
load is reused across all 9 relations (matmuls 2..9 of a PSUM group
carry ldweights=False).  Chain matmuls run in fp8 e4m3 DoubleRow (two
128-row contraction sub-tiles per instruction, 2x the bf16 PE rate);
adjacency is pre-scaled x512 and T re-scaled per step to sit in fp8's
dynamic range, with the scale ratios folded into the mixing
coefficients.  The product (A_a @ T)^T lands with the feature axis on
partitions, so the relation mix is a single fused scalar_tensor_tensor
per product, and the final step's output is exactly attn^T — the W0
matmul of the tail consumes it with no transpose anywhere in the chain.
End-to-end rel err ~2.5e-3 vs the f32 reference (chain quantization is
strongly attenuated by the large residual x in the output).

Sharding (8 cores): core c -> (b = c>>2, q = c&3): batch x n-quarter.
Each core holds A^T[b, :, :, q-slice] fp8 (2.25MB, SBUF-resident) and
computes all 8 heads (full 256-wide f) for its 256-row quarter.  The
per-step exchange is split by feature half: each 128-feature half of
the step's output is transposed back to row-major, quantized to fp8
(32KB) and AllGathered within the 4-core batch group while the other
half computes; the next step's feature-half passes each depend only on
their own half's gather.  A dummy warmup collective runs under the
input DMA to absorb the CC-core ramp.  inv(degree) is folded into the
W0 residual add as a per-partition scale.
"""

import sys

try:
    import concourse.bass as bass  # noqa: F401
except ImportError:  # pragma: no cover
    for _p in ("/opt/trn_rl_repo", "/root/.axon_site/_ro/trn_rl_repo"):
        if _p not in sys.path:
            sys.path.insert(0, _p)
    import concourse.bass as bass  # noqa: F401

import numpy as np
import ml_dtypes

import concourse.mybir as mybir
import concourse.tile as tile
from concourse import bacc
from concourse.bass_utils import run_bass_kernel_spmd

B, A, N, P, D, H = 2, 9, 1024, 4, 256, 8
DH = D // H
EPS = 1e-12
NCORES = 8
NQ = N // 4          # n-quarter per core
MC = N // 128        # n row-chunks

F32 = mybir.dt.float32
F32R = mybir.dt.float32r
BF16 = mybir.dt.bfloat16
F8 = mybir.dt.float8e4
ALU = mybir.AluOpType
ACTF = mybir.ActivationFunctionType
NPBF16 = ml_dtypes.bfloat16
NPF8 = ml_dtypes.float8_e4m3

ASCALE = 512.0                       # adjacency fp8 pre-scale
TSCALE = [1.0, 512.0, 512.0, 512.0]  # T fp8 storage scale per step

_CACHE: dict = {}


def _build_nc(chain_mode: str = "dr8", shared_exout: bool = False, no_reuse: bool = False,
              triv=(False, False, False, False)):
    """chain_mode: 'dr8' fp8+DoubleRow, 'f8' fp8 plain, 'bf16' bf16 chain."""
    fp8 = chain_mode in ("dr8", "f8")
    CDT = F8 if fp8 else BF16
    NPC = NPF8 if fp8 else NPBF16
    triv_aff2, triv_b1, triv_afff, triv_b2f = triv
    nc = bacc.Bacc("TRN2", target_bir_lowering=False, debug=False, num_devices=NCORES)

    dp = nc.declare_dram_parameter
    at_in = dp("at", [A, 128, MC * NQ], CDT, isOutput=False)      # A^T chunk-packed
    xt_in = dp("xt", [2, 128, N], BF16, isOutput=False)           # x[b]^T, d-chunked
    wv_in = dp("wv", [2, 128, D], BF16, isOutput=False)           # Wv_cat (all heads)
    bv_in = dp("bv", [MC, 128, D], BF16, isOutput=False)          # Bv_cat
    cfp_in = dp("cfp", [128, 2, P, A], F32, isOutput=False)       # per-partition coefs
    w0_in = dp("w0", [2, 128, D], BF16, isOutput=False)
    w1_in = dp("w1", [2, 128, 2 * D], BF16, isOutput=False)
    w2_in = dp("w2", [4, 128, D], BF16, isOutput=False)
    xtail_in = dp("xtail", [2, 128, D], F32, isOutput=False)      # x rows of quarter
    invd_in = dp("invd", [128, 2], F32, isOutput=False)           # inv degree cols
    g2_in = dp("g2", [128, D], F32, isOutput=False)
    b2_in = dp("b2", [128, D], F32, isOutput=False)
    gf_in = dp("gf", [128, 2 * D], F32, isOutput=False)
    bf_in = dp("bf", [128, 2 * D], F32, isOutput=False)
    b1_in = dp("b1", [128, 2 * D], F32, isOutput=False)
    b2f_in = dp("b2f", [128, D], F32, isOutput=False)
    id_in = dp("ident", [128, 128], BF16, isOutput=False)
    out_p = dp("out", [2, 128, D], F32, isOutput=True)

    ag4 = [[4 * g + i for i in range(4)] for g in range(NCORES // 4)]
    DR = mybir.MatmulPerfMode.DoubleRow
    WAVES = ((0, 3), (3, 6), (6, A))

    def ex_dram(name, shape):
        if shared_exout:
            return nc.dram_tensor(name, shape, CDT, addr_space="Shared")
        return nc.dram_tensor(name, shape, CDT)

    # The ldweights=False weight-reuse groups require the PE queue to
    # execute in exactly the emitted order (a reordered transpose or
    # matmul would clobber the loaded stationary).  Tile's scheduler
    # can reorder within an engine around stalls, so every PE
    # instruction carries an explicit ordering edge to its predecessor.
    _pe_prev = [None]

    def pe(mm):
        if _pe_prev[0] is not None:
            bass._add_dep_helper(mm.ins, _pe_prev[0].ins, sync=False,
                                 reason="pe-order")
        _pe_prev[0] = mm
        return mm

    with tile.TileContext(nc) as tc:
        with (
            tc.tile_pool(name="atp", bufs=A) as atp,
            tc.tile_pool(name="cst", bufs=1) as cst,
            tc.tile_pool(name="wrk", bufs=1) as wrk,
            tc.tile_pool(name="tt", bufs=2) as tt,
            tc.tile_pool(name="ps", bufs=5, space="PSUM") as ps,
            tc.tile_pool(name="psb", bufs=1, space="PSUM") as psb,
            tc.tile_pool(name="tp", bufs=2, space="PSUM") as tp,
        ):
            # warmup collective, issued before everything else: pays the
            # first-collective setup/ramp cost under the input DMA phase
            # instead of on the chain's first real AllGather.
            wu = wrk.tile([128, 16], F32, tag="wu")
            nc.vector.memset(wu[:], 0.0)
            wu_in = nc.dram_tensor("wu_in", [128, 16], F32)
            wu_out = nc.dram_tensor("wu_out", [4, 128, 16], F32)
            nc.sync.dma_start(wu_in[:], wu[:])
            nc.gpsimd.collective_compute(
                "AllGather", ALU.bypass, replica_groups=ag4,
                ins=[wu_in[:].opt()], outs=[wu_out[:].opt()])

            # ---- first wave of adjacency, then the V-proj inputs ----
            at8 = [None] * A
            for a in range(3):
                t = atp.tile([128, MC, NQ], CDT, tag="AT", name=f"at8{a}")
                nc.sync.dma_start(t[:].rearrange("p m q -> p (m q)"), at_in[a])
                at8[a] = t
            xt = cst.tile([128, 2, N], BF16)
            nc.sync.dma_start(xt[:], xt_in.rearrange("c p f -> p c f"))
            wv = cst.tile([128, 2, D], BF16)
            nc.sync.dma_start(wv[:], wv_in.rearrange("c p f -> p c f"))
            cfp = cst.tile([128, 2, P, A], F32)
            nc.sync.dma_start(cfp[:], cfp_in[:])
            identb = cst.tile([128, 128], BF16)
            nc.sync.dma_start(identb[:], id_in[:])
            bv = cst.tile([128, MC, D], BF16)
            for m in range(MC):
                nc.sync.dma_start(bv[:, m, :], bv_in[m])


            # ---- rest of adjacency (SBUF-resident all 4 steps) ----
            for a in range(3, A):
                t = atp.tile([128, MC, NQ], CDT, tag="AT", name=f"at8{a}")
                nc.sync.dma_start(t[:].rearrange("p m q -> p (m q)"), at_in[a])
                at8[a] = t

            # ---- remaining constants ----
            w0 = cst.tile([128, 2, D], BF16)
            nc.sync.dma_start(w0[:], w0_in.rearrange("c p f -> p c f"))
            w1 = cst.tile([128, 2, 2 * D], BF16)
            nc.sync.dma_start(w1[:], w1_in.rearrange("c p f -> p c f"))
            w2 = cst.tile([128, 4, D], BF16)
            nc.sync.dma_start(w2[:], w2_in.rearrange("c p f -> p c f"))
            xtl = cst.tile([128, 2, D], F32)
            nc.sync.dma_start(xtl[:], xtail_in.rearrange("c p f -> p c f"))
            invd = cst.tile([128, 2], F32)
            nc.sync.dma_start(invd[:], invd_in[:])
            g2b = cst.tile([128, D], F32)
            nc.sync.dma_start(g2b[:], g2_in[:])
            b2b = cst.tile([128, D], F32)
            nc.sync.dma_start(b2b[:], b2_in[:])
            gfb = cst.tile([128, 2 * D], F32)
            nc.sync.dma_start(gfb[:], gf_in[:])
            bfb = cst.tile([128, 2 * D], F32)
            nc.sync.dma_start(bfb[:], bf_in[:])
            b1b = cst.tile([128, 2 * D], F32)
            nc.sync.dma_start(b1b[:], b1_in[:])
            b2fb = cst.tile([128, D], F32)
            nc.sync.dma_start(b2fb[:], b2f_in[:])
            epst = cst.tile([128, 1], F32)
            nc.vector.memset(epst[:], EPS)
            # preload the Gelu/Sqrt activation tables under the input DMA
            warm = wrk.tile([128, 1], F32, tag="warm")
            nc.scalar.activation(warm[:], epst[:], ACTF.Gelu)
            nc.scalar.activation(warm[:], epst[:], ACTF.Sqrt)

            # ---- V = x @ Wv_cat + Bv -> T0 chunks ([n, f], fp8) ----
            # T layout is feature-half-major: [p, fc, chunk, 128f], so the
            # post-AllGather scatter lands as one DMA with 1KB lines.
            tcur = tt.tile([128, 2, MC, 128], CDT, tag="T")
            for m in range(MC):
                pv = ps.tile([128, D], F32, tag="pa")
                for dc in range(2):
                    pe(nc.tensor.matmul(
                        pv[:], xt[:, dc, m * 128:(m + 1) * 128],
                        wv[:, dc, :], start=(dc == 0), stop=(dc == 1)))
                # fp8 quantize fuses the Bv add
                for fc in range(2):
                    fsl = slice(fc * 128, (fc + 1) * 128)
                    nc.vector.tensor_add(tcur[:, fc, m, :], pv[:, fsl],
                                         bv[:, m, fsl])

            # ---- chain: 4 steps of T <- sum_a cmix[s,a] * (A_a @ T) ----
            def emit_wave(s, fc, a_lo, a_hi, acc_e, acc_o, tcur):
                pas = {}
                if chain_mode == "dr8":
                    for k in range(MC // 2):
                        for a in range(a_lo, a_hi):
                            if k == 0:
                                pas[a] = ps.tile([128, NQ], F32, tag="pa",
                                                 name=f"pa{s}{fc}{a}")
                            mm = pe(nc.tensor.matmul(
                                pas[a][:], tcur[:, fc, 2 * k:2 * k + 2, :],
                                at8[a][:, 2 * k:2 * k + 2, :],
                                start=(k == 0), stop=(k == MC // 2 - 1),
                                perf_mode=DR))
                            if a != a_lo and not no_reuse:
                                mm.ins.ldweights = False
                else:
                    for k in range(MC):
                        for a in range(a_lo, a_hi):
                            if k == 0:
                                pas[a] = ps.tile([128, NQ], F32, tag="pa",
                                                 name=f"pa{s}{fc}{a}")
                            mm = pe(nc.tensor.matmul(
                                pas[a][:], tcur[:, fc, k, :], at8[a][:, k, :],
                                start=(k == 0), stop=(k == MC - 1)))
                            if a != a_lo and not no_reuse:
                                mm.ins.ldweights = False
                # fused mix: acc += pa * c[s,a,f] (two alternating
                # accumulator chains for DVE ILP)
                for a in range(a_lo, a_hi):
                    acc = acc_e if a % 2 == 0 else acc_o
                    sc = cfp[:, fc, s, a:a + 1]
                    if a < 2:
                        nc.vector.tensor_scalar_mul(acc[:], pas[a][:], sc)
                    else:
                        nc.vector.scalar_tensor_tensor(
                            acc[:], pas[a][:], sc, acc[:],
                            op0=ALU.mult, op1=ALU.add)

            def emit_finish(s, fc, acc_e, acc_o, tnext):
                # combine accumulators (bf16), transpose own chunks back to
                # row-major, quantize to fp8 on the Scalar engine
                accb = wrk.tile([128, NQ], BF16, tag=f"accb{fc}")
                nc.vector.tensor_add(accb[:], acc_e[:], acc_o[:])
                exg = wrk.tile([128, 2, 128], CDT, tag=f"exg{fc}")
                for c in range(2):
                    ptr = tp.tile([128, 128], BF16, tag="tp", name=f"ptr{fc}")
                    pe(nc.tensor.transpose(
                        ptr[:], accb[:, c * 128:(c + 1) * 128], identb[:]))
                    nc.scalar.activation(exg[:, c, :], ptr[:], ACTF.Copy)
                exin = nc.dram_tensor(f"exi{s}{fc}", [128, 2, 128], CDT)
                exout = ex_dram(f"exo{s}{fc}", [4, 128, 2, 128])
                nc.sync.dma_start(exin[:], exg[:])
                nc.gpsimd.collective_compute(
                    "AllGather", ALU.bypass, replica_groups=ag4,
                    ins=[exin[:].opt()], outs=[exout[:].opt()])
                # one scatter DMA: dest [p, slot(g,c), f] is contiguous
                # 1KB per partition in the fc plane of tnext
                nc.sync.dma_start(
                    tnext[:, fc].rearrange("p (g c) f -> p g c f", g=4, c=2),
                    exout[:].rearrange("g p c f -> p g c f"))

            att_t = {}
            for s in range(P):
                last = s == P - 1
                tnext = None if last else tt.tile([128, 2, MC, 128], CDT,
                                                  tag="T")
                accs = {}
                for fc in range(2):
                    accs[fc] = (wrk.tile([128, NQ], F32, tag=f"acce{fc}",
                                         name=f"acce{s}{fc}"),
                                wrk.tile([128, NQ], F32, tag=f"acco{fc}",
                                         name=f"acco{s}{fc}"))
                # PE order: fc0 both waves, fc1 wave1, fc0's transposes
                # (its mix completes under fc1-wave1), fc1 wave2, fc1's
                # transposes — so each feature-half's AllGather launches
                # while the other half still computes.
                for w in WAVES:
                    emit_wave(s, 0, *w, *accs[0], tcur)
                emit_wave(s, 1, *WAVES[0], *accs[1], tcur)
                if last:
                    accb0 = wrk.tile([128, NQ], BF16, tag="accb0")
                    nc.vector.tensor_add(accb0[:], accs[0][0][:],
                                         accs[0][1][:])
                    att_t[0] = accb0
                else:
                    emit_finish(s, 0, *accs[0], tnext)
                for w in WAVES[1:]:
                    emit_wave(s, 1, *w, *accs[1], tcur)
                if last:
                    accb1 = wrk.tile([128, NQ], BF16, tag="accb1")
                    nc.vector.tensor_add(accb1[:], accs[1][0][:],
                                         accs[1][1][:])
                    att_t[1] = accb1
                else:
                    emit_finish(s, 1, *accs[1], tnext)
                    tcur = tnext

            # ---- tail for our 256-row n-quarter (2 chunks of 128) ----
            # att_t[fc] holds attn^T directly: [f-half, own 256 rows].
            # Stages are emitted i0/i1-interleaved so the two row-chunks'
            # serial LN chains overlap on the PE/DVE/Scalar queues.
            pr, resid, hb, ht, pf, g1, f2, f2t = {}, {}, {}, {}, {}, {}, {}, {}
            for i in range(2):
                pr[i] = ps.tile([128, D], F32, tag="pa", name=f"pr{i}")
                for fc in range(2):
                    pe(nc.tensor.matmul(
                        pr[i][:], att_t[fc][:, i * 128:(i + 1) * 128],
                        w0[:, fc, :], start=(fc == 0), stop=(fc == 1)))
            for i in range(2):
                # resid = pr * inv_deg + x   (inv(degree) folded in here)
                resid[i] = wrk.tile([128, D], F32, tag=f"resid{i}",
                                    name=f"resid{i}")
                nc.vector.scalar_tensor_tensor(
                    resid[i][:], pr[i][:], invd[:, i:i + 1], xtl[:, i, :],
                    op0=ALU.mult, op1=ALU.add)
            for i in range(2):
                # h = LayerNorm(resid) * gamma2 + beta2
                st = wrk.tile([128, 6], F32, tag=f"st{i}", name=f"st{i}")
                mv = wrk.tile([128, 2], F32, tag=f"mv{i}", name=f"mv{i}")
                nc.vector.bn_stats(st[:], resid[i][:])
                nc.vector.bn_aggr(mv[:], st[:])
                std = wrk.tile([128, 1], F32, tag=f"rstd{i}", name=f"rstd{i}")
                nc.scalar.activation(std[:], mv[:, 1:2], ACTF.Sqrt,
                                     bias=epst[:], scale=1.0)
                nc.vector.reciprocal(std[:], std[:])
                hb[i] = wrk.tile([128, D], BF16, tag=f"hb{i}", name=f"hb{i}")
                if triv_aff2:
                    # gamma2 == 1, beta2 == 0: normalize straight to bf16
                    nc.vector.tensor_scalar(hb[i][:], resid[i][:], mv[:, 0:1],
                                            std[:], op0=ALU.subtract,
                                            op1=ALU.mult)
                else:
                    hn = wrk.tile([128, D], F32, tag=f"hn{i}", name=f"hn{i}")
                    nc.vector.tensor_scalar(hn[:], resid[i][:], mv[:, 0:1],
                                            std[:], op0=ALU.subtract,
                                            op1=ALU.mult)
                    nc.gpsimd.tensor_mul(hn[:], hn[:], g2b[:])
                    nc.gpsimd.tensor_add(hb[i][:], hn[:], b2b[:])
            for i in range(2):
                # h^T for the W1 matmul
                ht[i] = wrk.tile([128, 2, 128], BF16, tag=f"ht{i}",
                                 name=f"ht{i}")
                for dc in range(2):
                    ptr = tp.tile([128, 128], BF16, tag="tp", name=f"tph{i}")
                    pe(nc.tensor.transpose(
                        ptr[:], hb[i][:, dc * 128:(dc + 1) * 128], identb[:]))
                    nc.scalar.activation(ht[i][:, dc, :], ptr[:], ACTF.Copy)
            for i in range(2):
                # f = gelu(h @ W1 + b1), then LayerNorm * gf + bf
                pf[i] = psb.tile([128, 2 * D], F32, tag="pf", name=f"pf{i}")
                for dc in range(2):
                    pe(nc.tensor.matmul(pf[i][:], ht[i][:, dc, :],
                                        w1[:, dc, :], start=(dc == 0),
                                        stop=(dc == 1)))
                g1[i] = wrk.tile([128, 2 * D], F32, tag=f"g1{i}",
                                 name=f"g1{i}")
                if triv_b1:
                    # b1 == 0: gelu straight from PSUM
                    nc.scalar.activation(g1[i][:], pf[i][:], ACTF.Gelu)
                else:
                    f1 = wrk.tile([128, 2 * D], F32, tag=f"f1{i}",
                                  name=f"f1{i}")
                    nc.vector.tensor_add(f1[:], pf[i][:], b1b[:])
                    nc.scalar.activation(g1[i][:], f1[:], ACTF.Gelu)
            for i in range(2):
                st2 = wrk.tile([128, 6], F32, tag=f"st2{i}", name=f"st2{i}")
                mv2 = wrk.tile([128, 2], F32, tag=f"mv2{i}", name=f"mv2{i}")
                nc.vector.bn_stats(st2[:], g1[i][:])
                nc.vector.bn_aggr(mv2[:], st2[:])
                std2 = wrk.tile([128, 1], F32, tag=f"rstd2{i}",
                                name=f"rstd2{i}")
                nc.scalar.activation(std2[:], mv2[:, 1:2], ACTF.Sqrt,
                                     bias=epst[:], scale=1.0)
                nc.vector.reciprocal(std2[:], std2[:])
                f2[i] = wrk.tile([128, 2 * D], BF16, tag=f"f2{i}",
                                 name=f"f2{i}")
                if triv_afff:
                    nc.vector.tensor_scalar(f2[i][:], g1[i][:], mv2[:, 0:1],
                                            std2[:], op0=ALU.subtract,
                                            op1=ALU.mult)
                else:
                    fn = wrk.tile([128, 2 * D], F32, tag=f"fn{i}",
                                  name=f"fn{i}")
                    nc.vector.tensor_scalar(fn[:], g1[i][:], mv2[:, 0:1],
                                            std2[:], op0=ALU.subtract,
                                            op1=ALU.mult)
                    nc.gpsimd.tensor_mul(fn[:], fn[:], gfb[:])
                    nc.gpsimd.tensor_add(f2[i][:], fn[:], bfb[:])
            for i in range(2):
                # f2^T, then out = f2 @ W2 + b2f + resid
                f2t[i] = wrk.tile([128, 4, 128], BF16, tag=f"f2t{i}",
                                  name=f"f2t{i}")
                for k in range(4):
                    ptr = tp.tile([128, 128], BF16, tag="tp", name=f"tpf{i}")
                    pe(nc.tensor.transpose(
                        ptr[:], f2[i][:, k * 128:(k + 1) * 128], identb[:]))
                    nc.scalar.activation(f2t[i][:, k, :], ptr[:], ACTF.Copy)
            for i in range(2):
                po = ps.tile([128, D], F32, tag="pa", name=f"po{i}")
                for k in range(4):
                    pe(nc.tensor.matmul(po[:], f2t[i][:, k, :], w2[:, k, :],
                                        start=(k == 0), stop=(k == 3)))
                ot = wrk.tile([128, D], F32, tag=f"ot{i}", name=f"ot{i}")
                if triv_b2f:
                    nc.vector.tensor_add(ot[:], po[:], resid[i][:])
                else:
                    nc.vector.tensor_add(ot[:], po[:], b2fb[:])
                    nc.vector.tensor_add(ot[:], ot[:], resid[i][:])
                nc.sync.dma_start(out_p[i], ot[:])

    nc.finalize()
    return nc


def _softmax_relu(kernels):
    r = np.maximum(kernels, 0.0)
    e = np.exp(r - r.max(axis=1, keepdims=True))
    return (e / e.sum(axis=1, keepdims=True)).astype(np.float32)  # [H, A, P]


def _prep_in_maps(adjacency, degree, x, kernels, Wv, Bv, W0, gamma2, beta2,
                  W1, b1, gf, bf, W2, b2f, chain_mode: str = "dr8"):
    fp8 = chain_mode in ("dr8", "f8")
    cdt = NPF8 if fp8 else NPBF16
    ascale = ASCALE if fp8 else 1.0
    tsc = TSCALE if fp8 else [1.0] * P

    soft = _softmax_relu(np.asarray(kernels, np.float32))
    wv_cat = np.ascontiguousarray(
        np.transpose(np.asarray(Wv, np.float32), (1, 0, 2)).reshape(D, D))
    bv_cat = np.transpose(np.asarray(Bv, np.float32), (1, 0, 2)).reshape(N, D)
    bv_r = (bv_cat.reshape(MC, 128, D) * tsc[0]).astype(NPBF16)
    invd_full = 1.0 / np.diagonal(np.asarray(degree, np.float32),
                                  axis1=1, axis2=2)  # [B, N]
    eye = np.eye(128, dtype=NPBF16)
    ones128 = np.ones((128, 1), np.float32)

    g2 = ones128 * np.asarray(gamma2, np.float32)[None, :]
    b2 = ones128 * np.asarray(beta2, np.float32)[None, :]
    gfB = ones128 * np.asarray(gf, np.float32)[None, :]
    bfB = ones128 * np.asarray(bf, np.float32)[None, :]
    b1B = ones128 * np.asarray(b1, np.float32)[None, :]
    b2fB = ones128 * np.asarray(b2f, np.float32)[None, :]
    w0r = np.asarray(W0, np.float32).reshape(2, 128, D).astype(NPBF16)
    w1r = np.asarray(W1, np.float32).reshape(2, 128, 2 * D).astype(NPBF16)
    w2r = np.asarray(W2, np.float32).reshape(4, 128, D).astype(NPBF16)

    # mix coefficients: chain step s applies soft[:, :, P-1-s]; fold the
    # adjacency fp8 pre-scale and the per-step T storage scales in.  The
    # final step folds T's storage scale out (output at true scale).
    hidx = np.arange(D) // DH
    tsc_out = list(tsc[1:]) + [1.0]
    cmix = np.empty((P, A, D), np.float32)
    for s in range(P):
        cmix[s] = (soft[hidx, :, P - 1 - s].T
                   * (tsc_out[s] / (tsc[s] * ascale)))
    # per-partition layout: cfp[p, fc, s, a] = cmix[s, a, fc*128 + p]
    cfp = np.ascontiguousarray(
        cmix.transpose(2, 0, 1).reshape(2, 128, P, A).transpose(1, 0, 2, 3))

    adjacency = np.asarray(adjacency, np.float32)
    x = np.asarray(x, np.float32)

    in_maps = []
    for c in range(NCORES):
        b = c >> 2
        q = c & 3
        nsl = slice(q * NQ, (q + 1) * NQ)

        # at[a, p, m*NQ+j] = adjacency[b, a, q*NQ+j, m*128+p] * ascale
        at_t = adjacency[b].transpose(0, 2, 1)[:, :, nsl]       # [A, n, NQ]
        at_c = np.ascontiguousarray(
            at_t.reshape(A, MC, 128, NQ).transpose(0, 2, 1, 3)
            .reshape(A, 128, MC * NQ)) * ascale
        xt_c = np.ascontiguousarray(x[b].T).reshape(2, 128, N).astype(NPBF16)
        xtail_c = np.ascontiguousarray(x[b, nsl]).reshape(2, 128, D)
        invd_c = np.ascontiguousarray(
            invd_full[b, nsl].reshape(2, 128).T)                 # [128, 2]

        in_maps.append({
            "at": at_c.astype(cdt),
            "xt": xt_c,
            "wv": wv_cat.reshape(2, 128, D).astype(NPBF16),
            "bv": bv_r,
            "cfp": cfp,
            "w0": w0r, "w1": w1r, "w2": w2r,
            "xtail": xtail_c,
            "invd": invd_c.astype(np.float32),
            "g2": g2, "b2": b2, "gf": gfB, "bf": bfB, "b1": b1B, "b2f": b2fB,
            "ident": eye,
        })
    return in_maps


def _triv_flags(inputs):
    g2 = np.asarray(inputs["gamma2"]); b2 = np.asarray(inputs["beta2"])
    gf_ = np.asarray(inputs["gf"]); bf_ = np.asarray(inputs["bf"])
    return (bool(np.all(g2 == 1) and np.all(b2 == 0)),
            bool(np.all(np.asarray(inputs["b1"]) == 0)),
            bool(np.all(gf_ == 1) and np.all(bf_ == 0)),
            bool(np.all(np.asarray(inputs["b2f"]) == 0)))


def _get_nc(inputs):
    triv = _triv_flags(inputs)
    key = ("nc", triv)
    if key not in _CACHE:
        _CACHE[key] = _build_nc(triv=triv)
    return _CACHE[key]


def kernel(**inputs) -> np.ndarray:
    nc = _get_nc(inputs)
    in_maps = _prep_in_maps(**inputs)
    res = run_bass_kernel_spmd(nc, in_maps, core_ids=list(range(NCORES)))
    out = np.empty((B, N, D), np.float32)
    for c in range(NCORES):
        b, q = c >> 2, c & 3
        out[b, q * NQ:(q + 1) * NQ] = res.results[c]["out"].reshape(NQ, D)
    return out


# revision 25
# speedup vs baseline: 1.1793x; 1.0134x over previous
"""MetaPathTransformer Trainium2 kernel (8 NeuronCores, Bass/Tile).

Math: the reference computes heads = inv(D) @ (M0@M1@M2@M3) @ V per
(head, batch), with M_i = sum_a soft[h,a,i] * adjacency[b,a] and D the
(diagonal-by-construction) degree matrix.  The chain is reassociated
right-to-left so every step is [N,N]@[N,256] instead of [N,N]@[N,N]:
per step, T' = sum_a c[s,a,f] * (A_a @ T), evaluated as 9 PE products
mixed on DVE.

PE orientation: T chunk-pairs are the STATIONARY operand and the
SBUF-resident A^T chunks are the MOVING operand, so each stationary
load is reused across all 9 relations (matmuls 2..9 of a PSUM group
carry ldweights=False).  Chain matmuls run in fp8 e4m3 DoubleRow (two
128-row contraction sub-tiles per instruction, 2x the bf16 PE rate);
adjacency is pre-scaled x512 and T re-scaled per step to sit in fp8's
dynamic range, with the scale ratios folded into the mixing
coefficients.  The product (A_a @ T)^T lands with the feature axis on
partitions, so the relation mix is a single fused scalar_tensor_tensor
per product, and the final step's output is exactly attn^T — the W0
matmul of the tail consumes it with no transpose anywhere in the chain.
End-to-end rel err ~2.5e-3 vs the f32 reference (chain quantization is
strongly attenuated by the large residual x in the output).

Sharding (8 cores): core c -> (b = c>>2, q = c&3): batch x n-quarter.
Each core holds A^T[b, :, :, q-slice] fp8 (2.25MB, SBUF-resident) and
computes all 8 heads (full 256-wide f) for its 256-row quarter.  The
per-step exchange is split by feature half: each 128-feature half of
the step's output is transposed back to row-major, quantized to fp8
(32KB) and AllGathered within the 4-core batch group while the other
half computes; the next step's feature-half passes each depend only on
their own half's gather.  A dummy warmup collective runs under the
input DMA to absorb the CC-core ramp.  inv(degree) is folded into the
W0 residual add as a per-partition scale.
"""

import sys

try:
    import concourse.bass as bass  # noqa: F401
except ImportError:  # pragma: no cover
    for _p in ("/opt/trn_rl_repo", "/root/.axon_site/_ro/trn_rl_repo"):
        if _p not in sys.path:
            sys.path.insert(0, _p)
    import concourse.bass as bass  # noqa: F401

import numpy as np
import ml_dtypes

import concourse.mybir as mybir
import concourse.tile as tile
from concourse import bacc
from concourse.bass_utils import run_bass_kernel_spmd

B, A, N, P, D, H = 2, 9, 1024, 4, 256, 8
DH = D // H
EPS = 1e-12
NCORES = 8
NQ = N // 4          # n-quarter per core
MC = N // 128        # n row-chunks

F32 = mybir.dt.float32
F32R = mybir.dt.float32r
BF16 = mybir.dt.bfloat16
F8 = mybir.dt.float8e4
ALU = mybir.AluOpType
ACTF = mybir.ActivationFunctionType
NPBF16 = ml_dtypes.bfloat16
NPF8 = ml_dtypes.float8_e4m3

ASCALE = 512.0                       # adjacency fp8 pre-scale
TSCALE = [1.0, 512.0, 512.0, 512.0]  # T fp8 storage scale per step

_CACHE: dict = {}


def _build_nc(chain_mode: str = "dr8", shared_exout: bool = False, no_reuse: bool = False,
              triv=(False, False, False, False)):
    """chain_mode: 'dr8' fp8+DoubleRow, 'f8' fp8 plain, 'bf16' bf16 chain."""
    fp8 = chain_mode in ("dr8", "f8")
    CDT = F8 if fp8 else BF16
    NPC = NPF8 if fp8 else NPBF16
    triv_aff2, triv_b1, triv_afff, triv_b2f = triv
    nc = bacc.Bacc("TRN2", target_bir_lowering=False, debug=False, num_devices=NCORES)

    dp = nc.declare_dram_parameter
    at_in = dp("at", [A, 128, MC * NQ], CDT, isOutput=False)      # A^T chunk-packed
    xt_in = dp("xt", [2, 128, N], BF16, isOutput=False)           # x[b]^T, d-chunked
    wv_in = dp("wv", [2, 128, D], BF16, isOutput=False)           # Wv_cat (all heads)
    bv_in = dp("bv", [MC, 128, D], BF16, isOutput=False)          # Bv_cat
    cfp_in = dp("cfp", [128, 2, P, A], F32, isOutput=False)       # per-partition coefs
    w0_in = dp("w0", [2, 128, D], BF16, isOutput=False)
    w1_in = dp("w1", [2, 128, 2 * D], BF16, isOutput=False)
    w2_in = dp("w2", [4, 128, D], BF16, isOutput=False)
    xtail_in = dp("xtail", [2, 128, D], F32, isOutput=False)      # x rows of quarter
    invd_in = dp("invd", [128, 2], F32, isOutput=False)           # inv degree cols
    g2_in = dp("g2", [128, D], F32, isOutput=False)
    b2_in = dp("b2", [128, D], F32, isOutput=False)
    gf_in = dp("gf", [128, 2 * D], F32, isOutput=False)
    bf_in = dp("bf", [128, 2 * D], F32, isOutput=False)
    b1_in = dp("b1", [128, 2 * D], F32, isOutput=False)
    b2f_in = dp("b2f", [128, D], F32, isOutput=False)
    id_in = dp("ident", [128, 128], BF16, isOutput=False)
    out_p = dp("out", [2, 128, D], F32, isOutput=True)

    ag4 = [[4 * g + i for i in range(4)] for g in range(NCORES // 4)]
    DR = mybir.MatmulPerfMode.DoubleRow
    WAVES = ((0, 3), (3, 6), (6, A))

    def ex_dram(name, shape):
        if shared_exout:
            return nc.dram_tensor(name, shape, CDT, addr_space="Shared")
        return nc.dram_tensor(name, shape, CDT)

    # The ldweights=False weight-reuse groups require the PE queue to
    # execute in exactly the emitted order (a reordered transpose or
    # matmul would clobber the loaded stationary).  Tile's scheduler
    # can reorder within an engine around stalls, so every PE
    # instruction carries an explicit ordering edge to its predecessor.
    _pe_prev = [None]

    def pe(mm):
        if _pe_prev[0] is not None:
            bass._add_dep_helper(mm.ins, _pe_prev[0].ins, sync=False,
                                 reason="pe-order")
        _pe_prev[0] = mm
        return mm

    with tile.TileContext(nc) as tc:
        with (
            tc.tile_pool(name="atp", bufs=A) as atp,
            tc.tile_pool(name="cst", bufs=1) as cst,
            tc.tile_pool(name="wrk", bufs=1) as wrk,
            tc.tile_pool(name="tt", bufs=2) as tt,
            tc.tile_pool(name="ps", bufs=5, space="PSUM") as ps,
            tc.tile_pool(name="psb", bufs=1, space="PSUM") as psb,
            tc.tile_pool(name="tp", bufs=2, space="PSUM") as tp,
        ):
            # warmup collective, issued before everything else: pays the
            # first-collective setup/ramp cost under the input DMA phase
            # instead of on the chain's first real AllGather.
            wu = wrk.tile([128, 16], F32, tag="wu")
            nc.vector.memset(wu[:], 0.0)
            wu_in = nc.dram_tensor("wu_in", [128, 16], F32)
            wu_out = nc.dram_tensor("wu_out", [4, 128, 16], F32)
            nc.sync.dma_start(wu_in[:], wu[:])
            nc.gpsimd.collective_compute(
                "AllGather", ALU.bypass, replica_groups=ag4,
                ins=[wu_in[:].opt()], outs=[wu_out[:].opt()])

            # ---- first wave of adjacency, then the V-proj inputs ----
            at8 = [None] * A
            for a in range(3):
                t = atp.tile([128, MC, NQ], CDT, tag="AT", name=f"at8{a}")
                nc.sync.dma_start(t[:].rearrange("p m q -> p (m q)"), at_in[a])
                at8[a] = t
            xt = cst.tile([128, 2, N], BF16)
            nc.sync.dma_start(xt[:], xt_in.rearrange("c p f -> p c f"))
            wv = cst.tile([128, 2, D], BF16)
            nc.sync.dma_start(wv[:], wv_in.rearrange("c p f -> p c f"))
            cfp = cst.tile([128, 2, P, A], F32)
            nc.sync.dma_start(cfp[:], cfp_in[:])
            identb = cst.tile([128, 128], BF16)
            nc.sync.dma_start(identb[:], id_in[:])
            bv = cst.tile([128, MC, D], BF16)
            for m in range(MC):
                nc.sync.dma_start(bv[:, m, :], bv_in[m])


            # ---- rest of adjacency (SBUF-resident all 4 steps) ----
            for a in range(3, A):
                t = atp.tile([128, MC, NQ], CDT, tag="AT", name=f"at8{a}")
                nc.sync.dma_start(t[:].rearrange("p m q -> p (m q)"), at_in[a])
                at8[a] = t

            # ---- remaining constants ----
            w0 = cst.tile([128, 2, D], BF16)
            nc.sync.dma_start(w0[:], w0_in.rearrange("c p f -> p c f"))
            w1 = cst.tile([128, 2, 2 * D], BF16)
            nc.sync.dma_start(w1[:], w1_in.rearrange("c p f -> p c f"))
            w2 = cst.tile([128, 4, D], BF16)
            nc.sync.dma_start(w2[:], w2_in.rearrange("c p f -> p c f"))
            xtl = cst.tile([128, 2, D], F32)
            nc.sync.dma_start(xtl[:], xtail_in.rearrange("c p f -> p c f"))
            invd = cst.tile([128, 2], F32)
            nc.sync.dma_start(invd[:], invd_in[:])
            g2b = cst.tile([128, D], F32)
            nc.sync.dma_start(g2b[:], g2_in[:])
            b2b = cst.tile([128, D], F32)
            nc.sync.dma_start(b2b[:], b2_in[:])
            gfb = cst.tile([128, 2 * D], F32)
            nc.sync.dma_start(gfb[:], gf_in[:])
            bfb = cst.tile([128, 2 * D], F32)
            nc.sync.dma_start(bfb[:], bf_in[:])
            b1b = cst.tile([128, 2 * D], F32)
            nc.sync.dma_start(b1b[:], b1_in[:])
            b2fb = cst.tile([128, D], F32)
            nc.sync.dma_start(b2fb[:], b2f_in[:])
            epst = cst.tile([128, 1], F32)
            nc.vector.memset(epst[:], EPS)
            # preload the Gelu/Sqrt activation tables under the input DMA
            warm = wrk.tile([128, 1], F32, tag="warm")
            nc.scalar.activation(warm[:], epst[:], ACTF.Gelu)
            nc.scalar.activation(warm[:], epst[:], ACTF.Sqrt)
            # scratch operands for PE keep-warm matmuls: the PE clock
            # ramps down during collective waits and then runs ~2x slow
            # for ~3us; dummy matmuls across the known AllGather gaps
            # keep it at full speed (sized below the expected gap so
            # they never delay real work)
            dsrc = wrk.tile([128, 512], BF16, tag="dsrc")
            nc.vector.memset(dsrc[:], 0.0)

            def pe_warm(n, name):
                for j in range(n):
                    dps = psb.tile([128, 512], F32, tag="pf",
                                   name=f"dw{name}{j}")
                    pe(nc.tensor.matmul(dps[:], dsrc[:, 0:128], dsrc[:],
                                        start=True, stop=True))

            # ---- V = x @ Wv_cat + Bv -> T0 chunks ([n, f], fp8) ----
            # T layout is feature-half-major: [p, fc, chunk, 128f], so the
            # post-AllGather scatter lands as one DMA with 1KB lines.
            pe_warm(8, "pre")
            tcur = tt.tile([128, 2, MC, 128], CDT, tag="T")
            for m in range(MC):
                pv = ps.tile([128, D], F32, tag="pa")
                for dc in range(2):
                    pe(nc.tensor.matmul(
                        pv[:], xt[:, dc, m * 128:(m + 1) * 128],
                        wv[:, dc, :], start=(dc == 0), stop=(dc == 1)))
                # fp8 quantize fuses the Bv add
                for fc in range(2):
                    fsl = slice(fc * 128, (fc + 1) * 128)
                    nc.vector.tensor_add(tcur[:, fc, m, :], pv[:, fsl],
                                         bv[:, m, fsl])

            # ---- chain: 4 steps of T <- sum_a cmix[s,a] * (A_a @ T) ----
            def emit_wave(s, fc, a_lo, a_hi, acc_e, acc_o, tcur):
                pas = {}
                if chain_mode == "dr8":
                    for k in range(MC // 2):
                        for a in range(a_lo, a_hi):
                            if k == 0:
                                pas[a] = ps.tile([128, NQ], F32, tag="pa",
                                                 name=f"pa{s}{fc}{a}")
                            mm = pe(nc.tensor.matmul(
                                pas[a][:], tcur[:, fc, 2 * k:2 * k + 2, :],
                                at8[a][:, 2 * k:2 * k + 2, :],
                                start=(k == 0), stop=(k == MC // 2 - 1),
                                perf_mode=DR))
                            if a != a_lo and not no_reuse:
                                mm.ins.ldweights = False
                else:
                    for k in range(MC):
                        for a in range(a_lo, a_hi):
                            if k == 0:
                                pas[a] = ps.tile([128, NQ], F32, tag="pa",
                                                 name=f"pa{s}{fc}{a}")
                            mm = pe(nc.tensor.matmul(
                                pas[a][:], tcur[:, fc, k, :], at8[a][:, k, :],
                                start=(k == 0), stop=(k == MC - 1)))
                            if a != a_lo and not no_reuse:
                                mm.ins.ldweights = False
                # fused mix: acc += pa * c[s,a,f] (two alternating
                # accumulator chains for DVE ILP)
                for a in range(a_lo, a_hi):
                    acc = acc_e if a % 2 == 0 else acc_o
                    sc = cfp[:, fc, s, a:a + 1]
                    if a < 2:
                        nc.vector.tensor_scalar_mul(acc[:], pas[a][:], sc)
                    else:
                        nc.vector.scalar_tensor_tensor(
                            acc[:], pas[a][:], sc, acc[:],
                            op0=ALU.mult, op1=ALU.add)

            def emit_finish(s, fc, acc_e, acc_o, tnext):
                # combine accumulators (bf16), transpose own chunks back to
                # row-major, quantize to fp8 on the Scalar engine
                accb = wrk.tile([128, NQ], BF16, tag=f"accb{fc}")
                nc.vector.tensor_add(accb[:], acc_e[:], acc_o[:])
                exg = wrk.tile([128, 2, 128], CDT, tag=f"exg{fc}")
                for c in range(2):
                    ptr = tp.tile([128, 128], BF16, tag="tp", name=f"ptr{fc}")
                    pe(nc.tensor.transpose(
                        ptr[:], accb[:, c * 128:(c + 1) * 128], identb[:]))
                    nc.scalar.activation(exg[:, c, :], ptr[:], ACTF.Copy)
                exin = nc.dram_tensor(f"exi{s}{fc}", [128, 2, 128], CDT)
                exout = ex_dram(f"exo{s}{fc}", [4, 128, 2, 128])
                nc.sync.dma_start(exin[:], exg[:])
                nc.gpsimd.collective_compute(
                    "AllGather", ALU.bypass, replica_groups=ag4,
                    ins=[exin[:].opt()], outs=[exout[:].opt()])
                # one scatter DMA: dest [p, slot(g,c), f] is contiguous
                # 1KB per partition in the fc plane of tnext
                nc.sync.dma_start(
                    tnext[:, fc].rearrange("p (g c) f -> p g c f", g=4, c=2),
                    exout[:].rearrange("g p c f -> p g c f"))

            att_t = {}
            for s in range(P):
                last = s == P - 1
                tnext = None if last else tt.tile([128, 2, MC, 128], CDT,
                                                  tag="T")
                accs = {}
                for fc in range(2):
                    accs[fc] = (wrk.tile([128, NQ], F32, tag=f"acce{fc}",
                                         name=f"acce{s}{fc}"),
                                wrk.tile([128, NQ], F32, tag=f"acco{fc}",
                                         name=f"acco{s}{fc}"))
                # PE order: fc0 both waves, fc1 wave1, fc0's transposes
                # (its mix completes under fc1-wave1), fc1 wave2, fc1's
                # transposes — so each feature-half's AllGather launches
                # while the other half still computes.
                for w in WAVES:
                    emit_wave(s, 0, *w, *accs[0], tcur)
                emit_wave(s, 1, *WAVES[0], *accs[1], tcur)
                if last:
                    accb0 = wrk.tile([128, NQ], BF16, tag="accb0")
                    nc.vector.tensor_add(accb0[:], accs[0][0][:],
                                         accs[0][1][:])
                    att_t[0] = accb0
                else:
                    emit_finish(s, 0, *accs[0], tnext)
                for w in WAVES[1:]:
                    emit_wave(s, 1, *w, *accs[1], tcur)
                if last:
                    accb1 = wrk.tile([128, NQ], BF16, tag="accb1")
                    nc.vector.tensor_add(accb1[:], accs[1][0][:],
                                         accs[1][1][:])
                    att_t[1] = accb1
                else:
                    emit_finish(s, 1, *accs[1], tnext)
                    pe_warm(16 if s == 0 else 10, f"s{s}")
                    tcur = tnext

            # ---- tail for our 256-row n-quarter (2 chunks of 128) ----
            # att_t[fc] holds attn^T directly: [f-half, own 256 rows].
            # Stages are emitted i0/i1-interleaved so the two row-chunks'
            # serial LN chains overlap on the PE/DVE/Scalar queues.
            pr, resid, hb, ht, pf, g1, f2, f2t = {}, {}, {}, {}, {}, {}, {}, {}
            for i in range(2):
                pr[i] = ps.tile([128, D], F32, tag="pa", name=f"pr{i}")
                for fc in range(2):
                    (nc.tensor.matmul(
                        pr[i][:], att_t[fc][:, i * 128:(i + 1) * 128],
                        w0[:, fc, :], start=(fc == 0), stop=(fc == 1)))
            for i in range(2):
                # resid = pr * inv_deg + x   (inv(degree) folded in here)
                resid[i] = wrk.tile([128, D], F32, tag=f"resid{i}",
                                    name=f"resid{i}")
                nc.vector.scalar_tensor_tensor(
                    resid[i][:], pr[i][:], invd[:, i:i + 1], xtl[:, i, :],
                    op0=ALU.mult, op1=ALU.add)
            for i in range(2):
                # h = LayerNorm(resid) * gamma2 + beta2
                st = wrk.tile([128, 6], F32, tag=f"st{i}", name=f"st{i}")
                mv = wrk.tile([128, 2], F32, tag=f"mv{i}", name=f"mv{i}")
                nc.vector.bn_stats(st[:], resid[i][:])
                nc.vector.bn_aggr(mv[:], st[:])
                std = wrk.tile([128, 1], F32, tag=f"rstd{i}", name=f"rstd{i}")
                nc.scalar.activation(std[:], mv[:, 1:2], ACTF.Sqrt,
                                     bias=epst[:], scale=1.0)
                nc.vector.reciprocal(std[:], std[:])
                hb[i] = wrk.tile([128, D], BF16, tag=f"hb{i}", name=f"hb{i}")
                if triv_aff2:
                    # gamma2 == 1, beta2 == 0: normalize straight to bf16
                    nc.vector.tensor_scalar(hb[i][:], resid[i][:], mv[:, 0:1],
                                            std[:], op0=ALU.subtract,
                                            op1=ALU.mult)
                else:
                    hn = wrk.tile([128, D], F32, tag=f"hn{i}", name=f"hn{i}")
                    nc.vector.tensor_scalar(hn[:], resid[i][:], mv[:, 0:1],
                                            std[:], op0=ALU.subtract,
                                            op1=ALU.mult)
                    nc.gpsimd.tensor_mul(hn[:], hn[:], g2b[:])
                    nc.gpsimd.tensor_add(hb[i][:], hn[:], b2b[:])
            for i in range(2):
                # h^T for the W1 matmul
                ht[i] = wrk.tile([128, 2, 128], BF16, tag=f"ht{i}",
                                 name=f"ht{i}")
                for dc in range(2):
                    ptr = tp.tile([128, 128], BF16, tag="tp", name=f"tph{i}")
                    (nc.tensor.transpose(
                        ptr[:], hb[i][:, dc * 128:(dc + 1) * 128], identb[:]))
                    nc.scalar.activation(ht[i][:, dc, :], ptr[:], ACTF.Copy)
            for i in range(2):
                # f = gelu(h @ W1 + b1), then LayerNorm * gf + bf
                pf[i] = psb.tile([128, 2 * D], F32, tag="pf", name=f"pf{i}")
                for dc in range(2):
                    (nc.tensor.matmul(pf[i][:], ht[i][:, dc, :],
                                        w1[:, dc, :], start=(dc == 0),
                                        stop=(dc == 1)))
                g1[i] = wrk.tile([128, 2 * D], F32, tag=f"g1{i}",
                                 name=f"g1{i}")
                if triv_b1:
                    # b1 == 0: gelu straight from PSUM
                    nc.scalar.activation(g1[i][:], pf[i][:], ACTF.Gelu)
                else:
                    f1 = wrk.tile([128, 2 * D], F32, tag=f"f1{i}",
                                  name=f"f1{i}")
                    nc.vector.tensor_add(f1[:], pf[i][:], b1b[:])
                    nc.scalar.activation(g1[i][:], f1[:], ACTF.Gelu)
            for i in range(2):
                st2 = wrk.tile([128, 6], F32, tag=f"st2{i}", name=f"st2{i}")
                mv2 = wrk.tile([128, 2], F32, tag=f"mv2{i}", name=f"mv2{i}")
                nc.vector.bn_stats(st2[:], g1[i][:])
                nc.vector.bn_aggr(mv2[:], st2[:])
                std2 = wrk.tile([128, 1], F32, tag=f"rstd2{i}",
                                name=f"rstd2{i}")
                nc.scalar.activation(std2[:], mv2[:, 1:2], ACTF.Sqrt,
                                     bias=epst[:], scale=1.0)
                nc.vector.reciprocal(std2[:], std2[:])
                f2[i] = wrk.tile([128, 2 * D], BF16, tag=f"f2{i}",
                                 name=f"f2{i}")
                if triv_afff:
                    nc.vector.tensor_scalar(f2[i][:], g1[i][:], mv2[:, 0:1],
                                            std2[:], op0=ALU.subtract,
                                            op1=ALU.mult)
                else:
                    fn = wrk.tile([128, 2 * D], F32, tag=f"fn{i}",
                                  name=f"fn{i}")
                    nc.vector.tensor_scalar(fn[:], g1[i][:], mv2[:, 0:1],
                                            std2[:], op0=ALU.subtract,
                                            op1=ALU.mult)
                    nc.gpsimd.tensor_mul(fn[:], fn[:], gfb[:])
                    nc.gpsimd.tensor_add(f2[i][:], fn[:], bfb[:])
            for i in range(2):
                # f2^T, then out = f2 @ W2 + b2f + resid
                f2t[i] = wrk.tile([128, 4, 128], BF16, tag=f"f2t{i}",
                                  name=f"f2t{i}")
                for k in range(4):
                    ptr = tp.tile([128, 128], BF16, tag="tp", name=f"tpf{i}")
                    (nc.tensor.transpose(
                        ptr[:], f2[i][:, k * 128:(k + 1) * 128], identb[:]))
                    nc.scalar.activation(f2t[i][:, k, :], ptr[:], ACTF.Copy)
            for i in range(2):
                po = ps.tile([128, D], F32, tag="pa", name=f"po{i}")
                for k in range(4):
                    (nc.tensor.matmul(po[:], f2t[i][:, k, :], w2[:, k, :],
                                        start=(k == 0), stop=(k == 3)))
                ot = wrk.tile([128, D], F32, tag=f"ot{i}", name=f"ot{i}")
                if triv_b2f:
                    nc.vector.tensor_add(ot[:], po[:], resid[i][:])
                else:
                    nc.vector.tensor_add(ot[:], po[:], b2fb[:])
                    nc.vector.tensor_add(ot[:], ot[:], resid[i][:])
                nc.sync.dma_start(out_p[i], ot[:])

    nc.finalize()
    return nc


def _softmax_relu(kernels):
    r = np.maximum(kernels, 0.0)
    e = np.exp(r - r.max(axis=1, keepdims=True))
    return (e / e.sum(axis=1, keepdims=True)).astype(np.float32)  # [H, A, P]


def _prep_in_maps(adjacency, degree, x, kernels, Wv, Bv, W0, gamma2, beta2,
                  W1, b1, gf, bf, W2, b2f, chain_mode: str = "dr8"):
    fp8 = chain_mode in ("dr8", "f8")
    cdt = NPF8 if fp8 else NPBF16
    ascale = ASCALE if fp8 else 1.0
    tsc = TSCALE if fp8 else [1.0] * P

    soft = _softmax_relu(np.asarray(kernels, np.float32))
    wv_cat = np.ascontiguousarray(
        np.transpose(np.asarray(Wv, np.float32), (1, 0, 2)).reshape(D, D))
    bv_cat = np.transpose(np.asarray(Bv, np.float32), (1, 0, 2)).reshape(N, D)
    bv_r = (bv_cat.reshape(MC, 128, D) * tsc[0]).astype(NPBF16)
    invd_full = 1.0 / np.diagonal(np.asarray(degree, np.float32),
                                  axis1=1, axis2=2)  # [B, N]
    eye = np.eye(128, dtype=NPBF16)
    ones128 = np.ones((128, 1), np.float32)

    g2 = ones128 * np.asarray(gamma2, np.float32)[None, :]
    b2 = ones128 * np.asarray(beta2, np.float32)[None, :]
    gfB = ones128 * np.asarray(gf, np.float32)[None, :]
    bfB = ones128 * np.asarray(bf, np.float32)[None, :]
    b1B = ones128 * np.asarray(b1, np.float32)[None, :]
    b2fB = ones128 * np.asarray(b2f, np.float32)[None, :]
    w0r = np.asarray(W0, np.float32).reshape(2, 128, D).astype(NPBF16)
    w1r = np.asarray(W1, np.float32).reshape(2, 128, 2 * D).astype(NPBF16)
    w2r = np.asarray(W2, np.float32).reshape(4, 128, D).astype(NPBF16)

    # mix coefficients: chain step s applies soft[:, :, P-1-s]; fold the
    # adjacency fp8 pre-scale and the per-step T storage scales in.  The
    # final step folds T's storage scale out (output at true scale).
    hidx = np.arange(D) // DH
    tsc_out = list(tsc[1:]) + [1.0]
    cmix = np.empty((P, A, D), np.float32)
    for s in range(P):
        cmix[s] = (soft[hidx, :, P - 1 - s].T
                   * (tsc_out[s] / (tsc[s] * ascale)))
    # per-partition layout: cfp[p, fc, s, a] = cmix[s, a, fc*128 + p]
    cfp = np.ascontiguousarray(
        cmix.transpose(2, 0, 1).reshape(2, 128, P, A).transpose(1, 0, 2, 3))

    adjacency = np.asarray(adjacency, np.float32)
    x = np.asarray(x, np.float32)

    in_maps = []
    for c in range(NCORES):
        b = c >> 2
        q = c & 3
        nsl = slice(q * NQ, (q + 1) * NQ)

        # at[a, p, m*NQ+j] = adjacency[b, a, q*NQ+j, m*128+p] * ascale
        at_t = adjacency[b].transpose(0, 2, 1)[:, :, nsl]       # [A, n, NQ]
        at_c = np.ascontiguousarray(
            at_t.reshape(A, MC, 128, NQ).transpose(0, 2, 1, 3)
            .reshape(A, 128, MC * NQ)) * ascale
        xt_c = np.ascontiguousarray(x[b].T).reshape(2, 128, N).astype(NPBF16)
        xtail_c = np.ascontiguousarray(x[b, nsl]).reshape(2, 128, D)
        invd_c = np.ascontiguousarray(
            invd_full[b, nsl].reshape(2, 128).T)                 # [128, 2]

        in_maps.append({
            "at": at_c.astype(cdt),
            "xt": xt_c,
            "wv": wv_cat.reshape(2, 128, D).astype(NPBF16),
            "bv": bv_r,
            "cfp": cfp,
            "w0": w0r, "w1": w1r, "w2": w2r,
            "xtail": xtail_c,
            "invd": invd_c.astype(np.float32),
            "g2": g2, "b2": b2, "gf": gfB, "bf": bfB, "b1": b1B, "b2f": b2fB,
            "ident": eye,
        })
    return in_maps


def _triv_flags(inputs):
    g2 = np.asarray(inputs["gamma2"]); b2 = np.asarray(inputs["beta2"])
    gf_ = np.asarray(inputs["gf"]); bf_ = np.asarray(inputs["bf"])
    return (bool(np.all(g2 == 1) and np.all(b2 == 0)),
            bool(np.all(np.asarray(inputs["b1"]) == 0)),
            bool(np.all(gf_ == 1) and np.all(bf_ == 0)),
            bool(np.all(np.asarray(inputs["b2f"]) == 0)))


def _get_nc(inputs):
    triv = _triv_flags(inputs)
    key = ("nc", triv)
    if key not in _CACHE:
        _CACHE[key] = _build_nc(triv=triv)
    return _CACHE[key]


def kernel(**inputs) -> np.ndarray:
    nc = _get_nc(inputs)
    in_maps = _prep_in_maps(**inputs)
    res = run_bass_kernel_spmd(nc, in_maps, core_ids=list(range(NCORES)))
    out = np.empty((B, N, D), np.float32)
    for c in range(NCORES):
        b, q = c >> 2, c & 3
        out[b, q * NQ:(q + 1) * NQ] = res.results[c]["out"].reshape(NQ, D)
    return out
